# revision 1
# baseline (speedup 1.0000x reference)
"""Trainium2 Bass kernel for nn_BindingGNN (GATv2-style message-passing GNN).

Strategy (8 NeuronCores, SPMD, single NEFF):
  - Nodes partitioned by graph (2 graphs/core, edge-balanced pairing).
  - Edges owned by the core of their dst node, sorted by dst, packed into
    128-edge chunks grouped under 128-node-slot groups (uniform static
    structure across cores; per-core variation lives in input data only).
  - Per layer: xl table (all nodes) recomputed per-core from the AllGathered
    feature-major h; per-edge gather of xl rows via dma_gather; xr-expand and
    edge-feature projection via small selector matmuls (host-built 0/1
    blobs); attention softmax without max-subtraction (values are O(1),
    constant bias for range safety); segment-sum aggregation via selector
    matmuls accumulating in PSUM.
  - Cross-core comm: AllGather of the feature-major local h between layers.
  - Pooling/head computed fully locally (graphs are core-local).
Everything is fp16 on-chip with f32 PSUM/statistics.
"""
import sys
import numpy as np

sys.path.insert(0, "/opt/trn_rl_repo")

import concourse.bass as bass  # noqa: E402
import concourse.bacc as bacc  # noqa: E402
import concourse.tile as tile  # noqa: E402
from concourse import mybir  # noqa: E402
from concourse.masks import make_identity  # noqa: E402

F16 = mybir.dt.float16
F32 = mybir.dt.float32
I16 = mybir.dt.int16
AF = mybir.ActivationFunctionType
OP = mybir.AluOpType

HID = 256
NODE_DIM = 1280
L = 4
H = 4
DH = 64
EH = 16
B = 16
NCORES = 8
KX = 11  # ceil((1280+1)/128)
EXP_BIAS = -3.0
GATE_BIAS = -2.0


# ----------------------------------------------------------------------------
# host-side math (edge MLP is static per-edge preprocessing)
# ----------------------------------------------------------------------------
def _erf(x):
    # Abramowitz-Stegun is not accurate enough; use scipy if present else
    # a high-accuracy rational approx via numpy tanh-free formulation.
    try:
        from scipy.special import erf
        return erf(x)
    except Exception:
        import math
        v = np.vectorize(math.erf)
        return v(x).astype(x.dtype)


def _gelu_np(x):
    x64 = x.astype(np.float64)
    return (0.5 * x64 * (1.0 + _erf(x64 / np.sqrt(2.0)))).astype(np.float32)


def _edge_mlp_host(edge_attr, W_e1, b_e1, W_e2, b_e2):
    e = _gelu_np(edge_attr @ W_e1 + b_e1) @ W_e2 + b_e2
    return e.astype(np.float32)


# ----------------------------------------------------------------------------
# host-side sharding / blob construction
# ----------------------------------------------------------------------------
def prepare(inputs):
    x = np.asarray(inputs["x"], np.float32)
    edge_attr = np.asarray(inputs["edge_attr"], np.float32)
    edge_index = np.asarray(inputs["edge_index"]).astype(np.int64)
    batch = np.asarray(inputs["batch"]).astype(np.int64)
    N = x.shape[0]
    E = edge_index.shape[1]

    e_feat = _edge_mlp_host(edge_attr, np.asarray(inputs["W_e1"], np.float32),
                            np.asarray(inputs["b_e1"], np.float32),
                            np.asarray(inputs["W_e2"], np.float32),
                            np.asarray(inputs["b_e2"], np.float32))
    e_mean = e_feat.mean(0)

    # graph node ranges (batch is sorted)
    gcounts = np.bincount(batch, minlength=B)
    gstart = np.zeros(B + 1, np.int64)
    gstart[1:] = np.cumsum(gcounts)

    # weight per graph ~ edges into it + nodes (self-loops)
    dst_graph = batch[edge_index[1]]
    gedges = np.bincount(dst_graph, minlength=B) + gcounts
    order = np.argsort(-gedges, kind="stable")
    glist = [sorted([int(order[i]), int(order[B - 1 - i])]) for i in range(NCORES)]

    # per-core local node lists
    loc2glob = []
    for c in range(NCORES):
        ga, gb = glist[c]
        loc2glob.append(np.concatenate([np.arange(gstart[ga], gstart[ga + 1]),
                                        np.arange(gstart[gb], gstart[gb + 1])]))
    n_loc = np.array([len(v) for v in loc2glob])
    NLOC = int(-(-n_loc.max() // 128) * 128)
    NT = NLOC // 128
    GLOB = NCORES * NLOC
    assert GLOB < 32768, "padded node table must fit int16 indices"

    core_of = np.zeros(N, np.int64)
    slot_of = np.zeros(N, np.int64)
    for c in range(NCORES):
        core_of[loc2glob[c]] = c
        slot_of[loc2glob[c]] = np.arange(len(loc2glob[c]))
    padded_id = core_of * NLOC + slot_of  # global padded table row per node

    # ---- per-core edge lists (real edges + self-loops for all NLOC slots)
    core_edges = []  # (src_padded, dst_slot, efeat)
    for c in range(NCORES):
        sel = core_of[edge_index[1]] == c
        src_p = padded_id[edge_index[0][sel]]
        dst_s = slot_of[edge_index[1][sel]]
        ef = e_feat[sel]
        # self-loops for every slot (including padded slots: keeps den>0)
        sl_src = c * NLOC + np.arange(NLOC)
        sl_dst = np.arange(NLOC)
        sl_ef = np.broadcast_to(e_mean, (NLOC, EH))
        src_p = np.concatenate([src_p, sl_src])
        dst_s = np.concatenate([dst_s, sl_dst])
        ef = np.concatenate([ef, sl_ef], axis=0).astype(np.float32)
        o = np.argsort(dst_s, kind="stable")
        core_edges.append((src_p[o], dst_s[o], ef[o]))

    # ---- chunk structure: groups of 128 slots, CPG chunks per group
    CPG = 0
    for c in range(NCORES):
        dst_s = core_edges[c][1]
        gcnt = np.bincount(dst_s // 128, minlength=NT)
        CPG = max(CPG, int(-(-gcnt.max() // 128)))
    CPG += CPG % 2  # even, so chunk-pair DMA packing divides evenly
    NCH = NT * CPG
    SLOTS = NCH * 128

    per_core = []
    for c in range(NCORES):
        src_p, dst_s, ef = core_edges[c]
        M = len(src_p)
        grp = dst_s // 128
        # rank of edge within its group
        gcnt = np.bincount(grp, minlength=NT)
        goff = np.zeros(NT + 1, np.int64)
        goff[1:] = np.cumsum(gcnt)
        rank = np.arange(M) - goff[grp]
        pos = grp * (CPG * 128) + rank  # slot position in padded stream
        assert pos.max() < SLOTS

        srcs = np.zeros(SLOTS, np.int16)
        srcs[pos] = src_p.astype(np.int16)
        dsts = np.full(SLOTS, -1, np.int64)
        dsts[pos] = dst_s
        efs = np.zeros((SLOTS, EH), np.float32)
        efs[pos] = ef

        ch = np.arange(SLOTS) // 128
        ei = np.arange(SLOTS) % 128
        valid = dsts >= 0
        r = np.where(valid, dsts - (ch // CPG) * 128, 0)

        scb = np.zeros((NCH, 128, 128), np.float16)
        sctb = np.zeros((NCH, 128, 128), np.float16)
        scb[ch[valid], r[valid], ei[valid]] = 1.0
        sctb[ch[valid], ei[valid], r[valid]] = 1.0
        ecb = np.zeros((NCH, 17, 128), np.float16)
        ecb[:, :16, :] = efs.reshape(NCH, 128, EH).transpose(0, 2, 1).astype(np.float16)
        ecb[:, 16, :] = 1.0

        # pack pairs along free dim so per-partition DMA lines are 512B
        assert NCH % 2 == 0
        scb2 = scb.reshape(NCH // 2, 2, 128, 128).transpose(0, 2, 1, 3).reshape(NCH // 2, 128, 256)
        sctb2 = sctb.reshape(NCH // 2, 2, 128, 128).transpose(0, 2, 1, 3).reshape(NCH // 2, 128, 256)
        ecb2 = ecb.reshape(NCH // 2, 2, 17, 128).transpose(0, 2, 1, 3).reshape(NCH // 2, 17, 256)

        # dma_gather index layout: idx j at [j%16, j//16], replicated over 128 partitions
        idx16 = srcs.reshape(SLOTS // 16, 16).T  # [16, SLOTS//16]
        idx128 = np.tile(idx16, (8, 1)).astype(np.int16)

        # gmask: one-hot graph per slot
        gm = np.zeros((NLOC, 2), np.float16)
        ga, gb = glist[c]
        na = gstart[ga + 1] - gstart[ga]
        nb = gstart[gb + 1] - gstart[gb]
        gm[:na, 0] = 1.0
        gm[na:na + nb, 1] = 1.0
        gmask = gm.reshape(NT, 128, 2)

        # xT (padded transposed x for this core's slots)
        xT = np.zeros((KX * 128, NLOC), np.float16)
        xT[:NODE_DIM, :len(loc2glob[c])] = x[loc2glob[c]].T.astype(np.float16)
        xT[NODE_DIM, :] = 1.0  # bias row (applies +b_in; harmless for padding)

        per_core.append(dict(scb=scb2, sctb=sctb2, ecb=ecb2, idx=idx128,
                             gmask=gmask, xT=xT))
        per_core[-1]["_dbg"] = dict(srcs=srcs, dsts=dsts, efs=efs)

    # ---- shared weights
    f32 = np.float32
    W_in = np.asarray(inputs["W_in"], f32)
    b_in = np.asarray(inputs["b_in"], f32)
    winp = np.zeros((KX * 128, HID), f32)
    winp[:NODE_DIM] = W_in
    winp[NODE_DIM] = b_in
    winp = winp.reshape(KX, 128, HID).astype(np.float16)

    def rep(v):  # replicate a [HID] vector across partitions
        return np.broadcast_to(np.asarray(v, f32), (128, HID)).astype(np.float16).copy()

    Wl = np.asarray(inputs["Wl"], f32)
    Wr = np.asarray(inputs["Wr"], f32)
    bl = np.asarray(inputs["bl"], f32)
    br = np.asarray(inputs["br"], f32)
    We = np.asarray(inputs["We"], f32)
    att = np.asarray(inputs["att"], f32)
    bconv = np.asarray(inputs["bconv"], f32)
    ln_g = np.asarray(inputs["ln_g"], f32)
    ln_b = np.asarray(inputs["ln_b"], f32)

    wl = Wl.reshape(L, 2, 128, HID).astype(np.float16)
    wr = Wr.reshape(L, 2, 128, HID).astype(np.float16)
    weaug = np.zeros((L, 17, HID), f32)
    weaug[:, :16] = We
    weaug[:, 16] = bl + br
    weaug = weaug.astype(np.float16)
    attf = np.stack([rep(att[i].reshape(HID)) for i in range(L)])
    bconv2 = np.stack([rep(bconv[i] + bl[i]) for i in range(L)])
    lng = np.stack([rep(ln_g[i]) for i in range(L)])
    lnb = np.stack([rep(ln_b[i]) for i in range(L)])

    Wg1 = np.asarray(inputs["Wg1"], f32)  # (256,128)
    Wg2 = np.asarray(inputs["Wg2"], f32)  # (128,1)
    Wh1 = np.asarray(inputs["Wh1"], f32)  # (256,64)
    Wh2 = np.asarray(inputs["Wh2"], f32)  # (64,1)
    GW = Wg1.shape[1]
    HW1 = Wh1.shape[1]
    shared = dict(
        winp=winp,
        lnin_g=rep(inputs["ln_in_g"]), lnin_b=rep(inputs["ln_in_b"]),
        wl=wl, wr=wr, weaug=weaug, attf=attf, bconv2=bconv2, lng=lng, lnb=lnb,
        wg1=Wg1.reshape(2, 128, GW).astype(np.float16),
        bg1=np.asarray(inputs["bg1"], f32).reshape(1, GW).astype(np.float16),
        wg2=np.broadcast_to(Wg2.reshape(GW), (128, GW)).astype(np.float16).copy(),
        bg2=np.full((128, 1), float(np.asarray(inputs["bg2"]).reshape(())), f32),
        wh1=Wh1.reshape(2, 128, HW1).astype(np.float16),
        bh1=np.broadcast_to(np.asarray(inputs["bh1"], f32), (128, HW1)).astype(np.float16).copy(),
        wh2=np.broadcast_to(Wh2.reshape(HW1), (128, HW1)).astype(np.float16).copy(),
        bh2=np.full((128, 1), float(np.asarray(inputs["bh2"]).reshape(())), f32),
    )

    in_maps = []
    dbg = []
    for c in range(NCORES):
        m = dict(shared)
        m.update(per_core[c])
        dbg.append(m.pop("_dbg", None))
        in_maps.append({k: np.ascontiguousarray(v) for k, v in m.items()})

    meta = dict(NLOC=NLOC, NT=NT, CPG=CPG, NCH=NCH, SLOTS=SLOTS, GLOB=GLOB,
                glist=glist, GW=GW, HW1=HW1, in_maps=in_maps, dbg=dbg,
                loc2glob=loc2glob)
    return meta


# ----------------------------------------------------------------------------
# device program
# ----------------------------------------------------------------------------
def build(meta, num_devices=NCORES, nlayers=L):
    NLOC, NT, CPG, NCH = meta["NLOC"], meta["NT"], meta["CPG"], meta["NCH"]
    SLOTS, GW, HW1 = meta["SLOTS"], meta["GW"], meta["HW1"]
    ICOLS = SLOTS // 16
    SUP = 8  # chunks per supergather (dma_gather fails above 1024 idxs/call)
    NSUP = -(-NCH // SUP)

    nc = bacc.Bacc("TRN2", target_bir_lowering=False, debug=False,
                   enable_asserts=True, num_devices=num_devices)

    def din(name, shape, dt=F16):
        return nc.dram_tensor(name, list(shape), dt, kind="ExternalInput").ap()

    # inputs
    xT_d = din("xT", (KX * 128, NLOC))
    winp_d = din("winp", (KX, 128, HID))
    lnin_g_d = din("lnin_g", (128, HID))
    lnin_b_d = din("lnin_b", (128, HID))
    wl_d = din("wl", (L, 2, 128, HID))
    wr_d = din("wr", (L, 2, 128, HID))
    weaug_d = din("weaug", (L, 17, HID))
    attf_d = din("attf", (L, 128, HID))
    bconv2_d = din("bconv2", (L, 128, HID))
    lng_d = din("lng", (L, 128, HID))
    lnb_d = din("lnb", (L, 128, HID))
    scb_d = din("scb", (NCH // 2, 128, 256))
    sctb_d = din("sctb", (NCH // 2, 128, 256))
    ecb_d = din("ecb", (NCH // 2, 17, 256))
    idx_d = din("idx", (128, ICOLS), I16)
    gmask_d = din("gmask", (NT, 128, 2))
    wg1_d = din("wg1", (2, 128, GW))
    bg1_d = din("bg1", (1, GW))
    wg2_d = din("wg2", (128, GW))
    bg2_d = din("bg2", (128, 1), F32)
    wh1_d = din("wh1", (2, 128, HW1))
    bh1_d = din("bh1", (128, HW1))
    wh2_d = din("wh2", (128, HW1))
    bh2_d = din("bh2", (128, 1), F32)
    y_d = nc.dram_tensor("y", [2, 1], F32, kind="ExternalOutput").ap()

    # internal DRAM
    hloc_d = nc.dram_tensor("hloc", [NLOC, HID], F16).ap()
    bounce_d = nc.dram_tensor("bounce", [2, 128, NLOC], F16).ap()
    xl_d = nc.dram_tensor("xl_table", [NCORES * NLOC, HID], F16).ap()
    hTg_d = [nc.dram_tensor(f"hTg{i}", [NCORES, 2, 128, NLOC], F16,
                            addr_space="Shared").ap() for i in range(L)]

    rg = [list(range(num_devices))]

    with tile.TileContext(nc) as tc:
        import contextlib
        ctx = contextlib.ExitStack()
        with ctx:
            const = ctx.enter_context(tc.tile_pool(name="const", bufs=1))
            work = ctx.enter_context(tc.tile_pool(name="work", bufs=3))
            small = ctx.enter_context(tc.tile_pool(name="small", bufs=4))
            xtp = ctx.enter_context(tc.tile_pool(name="xtp", bufs=3))
            scp = ctx.enter_context(tc.tile_pool(name="scp", bufs=3))
            xlg_p = ctx.enter_context(tc.tile_pool(name="xlg", bufs=2))
            ps_mm = ctx.enter_context(tc.tile_pool(name="ps_mm", bufs=2, space="PSUM"))
            ps_ed = ctx.enter_context(tc.tile_pool(name="ps_ed", bufs=2, space="PSUM"))
            ps_ag = ctx.enter_context(tc.tile_pool(name="ps_ag", bufs=2, space="PSUM"))

            # ---------------- resident tiles
            winp_t = const.tile([128, KX, HID], F16)
            nc.sync.dma_start(out=winp_t[:], in_=winp_d.rearrange("k p f -> p k f"))
            lnin_g_t = const.tile([128, HID], F16)
            nc.sync.dma_start(out=lnin_g_t[:], in_=lnin_g_d[:])
            lnin_b_t = const.tile([128, HID], F16)
            nc.sync.dma_start(out=lnin_b_t[:], in_=lnin_b_d[:])
            wl_t = const.tile([128, L, 2, HID], F16)
            nc.sync.dma_start(out=wl_t[:], in_=wl_d.rearrange("l k p f -> p l k f"))
            wr_t = const.tile([128, L, 2, HID], F16)
            nc.sync.dma_start(out=wr_t[:], in_=wr_d.rearrange("l k p f -> p l k f"))
            weaug_t = const.tile([17, L, HID], F16)
            nc.sync.dma_start(out=weaug_t[:], in_=weaug_d.rearrange("l p f -> p l f"))
            attf_t = const.tile([128, L, HID], F16)
            nc.sync.dma_start(out=attf_t[:], in_=attf_d.rearrange("l p f -> p l f"))
            bconv2_t = const.tile([128, L, HID], F16)
            nc.sync.dma_start(out=bconv2_t[:], in_=bconv2_d.rearrange("l p f -> p l f"))
            lng_t = const.tile([128, L, HID], F16)
            nc.sync.dma_start(out=lng_t[:], in_=lng_d.rearrange("l p f -> p l f"))
            lnb_t = const.tile([128, L, HID], F16)
            nc.sync.dma_start(out=lnb_t[:], in_=lnb_d.rearrange("l p f -> p l f"))
            idx_t = const.tile([128, ICOLS], I16)
            nc.sync.dma_start(out=idx_t[:], in_=idx_d[:])
            gmask_t = const.tile([128, NT, 2], F16)
            nc.sync.dma_start(out=gmask_t[:], in_=gmask_d.rearrange("t p g -> p t g"))
            wg1_t = const.tile([128, 2, GW], F16)
            nc.sync.dma_start(out=wg1_t[:], in_=wg1_d.rearrange("k p f -> p k f"))
            bg1_t = const.tile([1, GW], F16)
            nc.sync.dma_start(out=bg1_t[:], in_=bg1_d[:])
            wg2_t = const.tile([128, GW], F16)
            nc.sync.dma_start(out=wg2_t[:], in_=wg2_d[:])
            bg2_t = const.tile([128, 1], F32)
            nc.sync.dma_start(out=bg2_t[:], in_=bg2_d[:])
            wh1_t = const.tile([128, 2, HW1], F16)
            nc.sync.dma_start(out=wh1_t[:], in_=wh1_d.rearrange("k p f -> p k f"))
            bh1_t = const.tile([128, HW1], F16)
            nc.sync.dma_start(out=bh1_t[:], in_=bh1_d[:])
            wh2_t = const.tile([128, HW1], F16)
            nc.sync.dma_start(out=wh2_t[:], in_=wh2_d[:])
            bh2_t = const.tile([128, 1], F32)
            nc.sync.dma_start(out=bh2_t[:], in_=bh2_d[:])

            h_res = const.tile([128, NT, HID + 1], F16)
            hT_loc = const.tile([128, 2, NLOC], F16)
            xr_t = const.tile([128, NT, HID], F16)
            ones1_t = const.tile([1, 128], F16)
            nc.vector.memset(ones1_t[:], 1.0)
            eps_t = const.tile([128, 1], F32)
            nc.vector.memset(eps_t[:], 1e-5)
            expb_t = const.tile([128, 1], F32)
            nc.vector.memset(expb_t[:], EXP_BIAS)
            gateb_t = const.tile([128, 1], F32)
            nc.vector.memset(gateb_t[:], GATE_BIAS)
            ident_t = const.tile([128, 128], F16)
            make_identity(nc, ident_t[:])
            for t in range(NT):
                nc.vector.memset(h_res[:, t, HID:HID + 1], 1.0)

            def refine_recip(r_ap, x_ap, shape, tag):
                # r <- r*(2 - x*r), one Newton step on a LUT seed
                t = small.tile(shape, F32, tag=tag)
                nc.vector.tensor_tensor(out=t[:], in0=x_ap, in1=r_ap, op=OP.mult)
                nc.vector.tensor_scalar(out=t[:], in0=t[:], scalar1=2.0,
                                        scalar2=-1.0, op0=OP.subtract, op1=OP.mult)
                nc.vector.tensor_tensor(out=r_ap, in0=r_ap, in1=t[:], op=OP.mult)

            def refine_rsqrt(r_ap, x_ap, shape, tag):
                # r <- 0.5*r*(3 - x*r*r)
                t = small.tile(shape, F32, tag=tag)
                nc.vector.tensor_tensor(out=t[:], in0=r_ap, in1=r_ap, op=OP.mult)
                nc.vector.tensor_tensor(out=t[:], in0=x_ap, in1=t[:], op=OP.mult)
                nc.vector.tensor_scalar(out=t[:], in0=t[:], scalar1=3.0,
                                        scalar2=-0.5, op0=OP.subtract, op1=OP.mult)
                nc.vector.tensor_tensor(out=r_ap, in0=r_ap, in1=t[:], op=OP.mult)

            # ---------------- LN helper: s_t fp16 [128,HID] + musum f32 -> dest
            def layernorm(s_t, musum, g_ap, b_ap, dest_ap, gelu_after=False):
                mu = small.tile([128, 1], F32, tag="mu")
                nc.vector.tensor_scalar(out=mu[:], in0=musum, scalar1=1.0 / HID,
                                        scalar2=None, op0=OP.mult)
                d_t = work.tile([128, HID], F16, tag="d")
                nc.vector.tensor_scalar(out=d_t[:], in0=s_t, scalar1=mu[:],
                                        scalar2=None, op0=OP.subtract)
                scr = work.tile([128, HID], F16, tag="scr")
                vs = small.tile([128, 1], F32, tag="vs")
                nc.vector.tensor_tensor(out=scr[:], in0=d_t[:], in1=d_t[:], op=OP.mult)
                nc.vector.tensor_reduce(out=vs[:], in_=scr[:],
                                        axis=mybir.AxisListType.X, op=OP.add)
                vx = small.tile([128, 1], F32, tag="vx")
                nc.vector.tensor_scalar(out=vx[:], in0=vs[:], scalar1=1.0 / HID,
                                        scalar2=None, op0=OP.mult)
                nc.vector.tensor_scalar(out=vx[:], in0=vx[:], scalar1=eps_t[:],
                                        scalar2=None, op0=OP.add)
                sd = small.tile([128, 1], F32, tag="sd")
                nc.scalar.activation(out=sd[:], in_=vx[:], func=AF.Ln)
                rstd = small.tile([128, 1], F32, tag="rstd")
                nc.scalar.activation(out=rstd[:], in_=sd[:], func=AF.Exp, scale=-0.5)
                refine_rsqrt(rstd[:], vx[:], [128, 1], "nsr")
                n_t = work.tile([128, HID], F16, tag="n")
                nc.vector.tensor_scalar(out=n_t[:], in0=d_t[:], scalar1=rstd[:],
                                        scalar2=None, op0=OP.mult)
                nc.vector.tensor_tensor(out=n_t[:], in0=n_t[:], in1=g_ap, op=OP.mult)
                if gelu_after:
                    nc.vector.tensor_tensor(out=n_t[:], in0=n_t[:], in1=b_ap, op=OP.add)
                    nc.scalar.activation(out=dest_ap, in_=n_t[:], func=AF.Gelu)
                else:
                    nc.vector.tensor_tensor(out=dest_ap, in0=n_t[:], in1=b_ap, op=OP.add)

            # ---------------- phase A: input projection (local nodes)
            for t2 in range((NT + 1) // 2):
                tcnt = min(2, NT - t2 * 2)
                xt_t = xtp.tile([128, KX, 2 * 128], F16, tag="xt")
                for k in range(KX):
                    nc.sync.dma_start(
                        out=xt_t[:, k, :tcnt * 128],
                        in_=xT_d[k * 128:(k + 1) * 128,
                                 t2 * 256:t2 * 256 + tcnt * 128])
                for j in range(tcnt):
                    t = t2 * 2 + j
                    ps = ps_mm.tile([128, HID], F32, tag="mmps")
                    for k in range(KX):
                        nc.tensor.matmul(out=ps[:], lhsT=xt_t[:, k, j * 128:(j + 1) * 128],
                                         rhs=winp_t[:, k, :], start=(k == 0),
                                         stop=(k == KX - 1))
                    s_t = work.tile([128, HID], F16, tag="s")
                    musum = small.tile([128, 1], F32, tag="musum")
                    nc.scalar.activation(out=s_t[:], in_=ps[:], func=AF.Copy,
                                         accum_out=musum[:])
                    layernorm(s_t[:], musum[:], lnin_g_t[:], lnin_b_t[:],
                              h_res[:, t, :HID], gelu_after=True)
                    nc.sync.dma_start(out=hloc_d[t * 128:(t + 1) * 128, :],
                                      in_=h_res[:, t, :HID])

            # ---------------- per layer
            def transpose_and_gather(layer):
                # hloc (node-major, HBM) -> hT_loc (feature-major, SBUF)
                for half in range(2):
                    nc.sync.dma_start(out=hT_loc[:, half, :],
                                      in_=hloc_d[:, half * 128:(half + 1) * 128],
                                      transpose=True)
                if layer < L:
                    nc.sync.dma_start(out=bounce_d.rearrange("h p n -> p h n"),
                                      in_=hT_loc[:])
                    nc.gpsimd.collective_compute(
                        "AllGather", OP.bypass, replica_groups=rg,
                        ins=[bounce_d[:]], outs=[hTg_d[layer][:]])

            transpose_and_gather(0)

            for i in range(nlayers):
                # xl table over all ranks; xr for local nodes
                for r in range(NCORES):
                    for t2 in range((NT + 1) // 2):
                        tcnt = min(2, NT - t2 * 2)
                        ht = xtp.tile([128, 2, 2 * 128], F16, tag="ht")
                        for half in range(2):
                            nc.sync.dma_start(
                                out=ht[:, half, :tcnt * 128],
                                in_=hTg_d[i][r, half, :,
                                             t2 * 256:t2 * 256 + tcnt * 128])
                        for j in range(tcnt):
                            t = t2 * 2 + j
                            ps = ps_mm.tile([128, HID], F32, tag="mmps")
                            for half in range(2):
                                nc.tensor.matmul(
                                    out=ps[:], lhsT=ht[:, half, j * 128:(j + 1) * 128],
                                    rhs=wl_t[:, i, half, :],
                                    start=(half == 0), stop=(half == 1))
                            xl_t = work.tile([128, HID], F16, tag="xlt")
                            nc.scalar.activation(out=xl_t[:], in_=ps[:], func=AF.Copy)
                            row = (r * NT + t) * 128
                            nc.sync.dma_start(out=xl_d[row:row + 128, :], in_=xl_t[:])
                for t in range(NT):
                    ps = ps_mm.tile([128, HID], F32, tag="mmps")
                    for half in range(2):
                        nc.tensor.matmul(out=ps[:],
                                         lhsT=hT_loc[:, half, t * 128:(t + 1) * 128],
                                         rhs=wr_t[:, i, half, :],
                                         start=(half == 0), stop=(half == 1))
                    nc.scalar.activation(out=xr_t[:, t, :], in_=ps[:], func=AF.Copy)

                # edge phase
                xlg_tiles = {}
                agg = None
                for chk in range(NCH):
                    s, joff = divmod(chk, SUP)
                    if joff == 0:
                        cnt = min(SUP, NCH - s * SUP)
                        xlg = xlg_p.tile([128, SUP, HID], F16, tag="xlg")
                        nc.gpsimd.dma_gather(
                            out_ap=xlg[:, :cnt, :], in_ap=xl_d[:, :],
                            idxs_ap=idx_t[:, s * (SUP * 8):s * (SUP * 8) + cnt * 8],
                            num_idxs=cnt * 128, num_idxs_reg=cnt * 128,
                            elem_size=HID)
                        xlg_tiles[s] = xlg
                    xlg = xlg_tiles[s]
                    g, cidx = divmod(chk, CPG)

                    if chk % 2 == 0:
                        sc2 = scp.tile([128, 256], F16, tag="sc2")
                        nc.sync.dma_start(out=sc2[:], in_=scb_d[chk // 2])
                        sct2 = scp.tile([128, 256], F16, tag="sct2")
                        nc.sync.dma_start(out=sct2[:], in_=sctb_d[chk // 2])
                        ec2 = scp.tile([17, 256], F16, tag="ec2")
                        nc.sync.dma_start(out=ec2[:], in_=ecb_d[chk // 2])
                        sc2_cur, sct2_cur, ec2_cur = sc2, sct2, ec2
                    half = (chk % 2) * 128

                    ps = ps_ed.tile([128, HID], F32, tag="edps")
                    nc.tensor.matmul(out=ps[:], lhsT=sc2_cur[:, half:half + 128],
                                     rhs=xr_t[:, g, :], start=True, stop=False)
                    nc.tensor.matmul(out=ps[:], lhsT=ec2_cur[:, half:half + 128],
                                     rhs=weaug_t[:, i, :], start=False, stop=True)
                    m_t = work.tile([128, HID], F16, tag="m")
                    nc.vector.tensor_tensor(out=m_t[:], in0=xlg[:, joff, :],
                                            in1=ps[:], op=OP.add)
                    lr_t = work.tile([128, HID], F16, tag="lr")
                    nc.scalar.activation(out=lr_t[:], in_=m_t[:], func=AF.Copy,
                                         scale=0.2)
                    nc.vector.tensor_tensor(out=m_t[:], in0=m_t[:], in1=lr_t[:],
                                            op=OP.max)
                    v_t = work.tile([128, HID], F16, tag="v")
                    nc.vector.tensor_tensor(out=v_t[:], in0=m_t[:],
                                            in1=attf_t[:, i, :], op=OP.mult)
                    a_t = small.tile([128, H], F32, tag="a")
                    nc.vector.tensor_reduce(
                        out=a_t[:], in_=v_t[:].rearrange("p (h d) -> p h d", d=DH),
                        axis=mybir.AxisListType.X, op=OP.add)
                    u_t = work.tile([128, HID + H], F16, tag="u")
                    nc.scalar.activation(out=u_t[:, HID:HID + H], in_=a_t[:],
                                         func=AF.Exp, bias=expb_t[:])
                    nc.vector.tensor_tensor(
                        out=u_t[:, :HID].rearrange("p (h d) -> p h d", d=DH),
                        in0=xlg[:, joff, :].rearrange("p (h d) -> p h d", d=DH),
                        in1=u_t[:, HID:HID + H].to_broadcast([128, H, DH]),
                        op=OP.mult)
                    if cidx == 0:
                        agg = ps_ag.tile([128, HID + H], F32, tag="agg")
                    nc.tensor.matmul(out=agg[:], lhsT=sct2_cur[:, half:half + 128],
                                     rhs=u_t[:], start=(cidx == 0),
                                     stop=(cidx == CPG - 1))

                    if cidx == CPG - 1:
                        rd = small.tile([128, H], F32, tag="rd")
                        nc.scalar.activation(out=rd[:], in_=agg[:, HID:HID + H],
                                             func=AF.Ln)
                        nc.scalar.activation(out=rd[:], in_=rd[:], func=AF.Exp,
                                             scale=-1.0)
                        refine_recip(rd[:], agg[:, HID:HID + H], [128, H], "nrd")
                        o_t = work.tile([128, HID], F16, tag="o")
                        nc.vector.tensor_tensor(
                            out=o_t[:].rearrange("p (h d) -> p h d", d=DH),
                            in0=agg[:, :HID].rearrange("p (h d) -> p h d", d=DH),
                            in1=rd[:].to_broadcast([128, H, DH]), op=OP.mult)
                        nc.vector.tensor_tensor(out=o_t[:], in0=o_t[:],
                                                in1=bconv2_t[:, i, :], op=OP.add)
                        nc.scalar.activation(out=o_t[:], in_=o_t[:], func=AF.Gelu)
                        s_t = work.tile([128, HID], F16, tag="s")
                        musum = small.tile([128, 1], F32, tag="musum")
                        nc.vector.tensor_tensor(out=s_t[:], in0=o_t[:],
                                                in1=h_res[:, g, :HID], op=OP.add)
                        nc.vector.tensor_reduce(out=musum[:], in_=s_t[:],
                                                axis=mybir.AxisListType.X, op=OP.add)
                        layernorm(s_t[:], musum[:], lng_t[:, i, :], lnb_t[:, i, :],
                                  h_res[:, g, :HID])
                        nc.sync.dma_start(out=hloc_d[g * 128:(g + 1) * 128, :],
                                          in_=h_res[:, g, :HID])
                transpose_and_gather(i + 1)

            # ---------------- pooling + head
            pool_ps = ps_mm.tile([2, HID + 1], F32, tag="mmps")
            for t in range(NT):
                g1 = ps_mm.tile([128, GW], F32, tag="mmps")
                for half in range(2):
                    nc.tensor.matmul(out=g1[:],
                                     lhsT=hT_loc[:, half, t * 128:(t + 1) * 128],
                                     rhs=wg1_t[:, half, :], start=(half == 0),
                                     stop=False)
                nc.tensor.matmul(out=g1[:], lhsT=ones1_t[:],
                                 rhs=bg1_t[:], start=False, stop=True)
                t_t = work.tile([128, GW], F16, tag="tt")
                nc.scalar.activation(out=t_t[:], in_=g1[:], func=AF.Tanh)
                scr = work.tile([128, GW], F16, tag="scr2")
                gate = small.tile([128, 1], F32, tag="gate")
                nc.vector.tensor_tensor(out=scr[:], in0=t_t[:], in1=wg2_t[:],
                                        op=OP.mult)
                nc.vector.tensor_reduce(out=gate[:], in_=scr[:],
                                        axis=mybir.AxisListType.X, op=OP.add)
                nc.vector.tensor_scalar(out=gate[:], in0=gate[:], scalar1=bg2_t[:],
                                        scalar2=None, op0=OP.add)
                eg = small.tile([128, 1], F16, tag="eg")
                nc.scalar.activation(out=eg[:], in_=gate[:], func=AF.Exp,
                                     bias=gateb_t[:])
                wm = small.tile([128, 2], F16, tag="wm")
                nc.vector.tensor_tensor(out=wm[:], in0=gmask_t[:, t, :],
                                        in1=eg[:].to_broadcast([128, 2]), op=OP.mult)
                nc.tensor.matmul(out=pool_ps[:], lhsT=wm[:], rhs=h_res[:, t, :],
                                 start=(t == 0), stop=(t == NT - 1))
            rd = small.tile([2, 1], F32, tag="prd")
            nc.scalar.activation(out=rd[:], in_=pool_ps[:, HID:HID + 1], func=AF.Ln)
            nc.scalar.activation(out=rd[:], in_=rd[:], func=AF.Exp, scale=-1.0)
            refine_recip(rd[:], pool_ps[:, HID:HID + 1], [2, 1], "npd")
            pooled = work.tile([2, HID], F16, tag="pooled")
            nc.vector.tensor_scalar(out=pooled[:], in0=pool_ps[:, :HID],
                                    scalar1=rd[:], scalar2=None, op0=OP.mult)
            pooledT = work.tile([128, 2, 2], F16, tag="pooledT")
            for half in range(2):
                tp = ps_mm.tile([128, 2], F16, tag="mmps")
                nc.tensor.transpose(out=tp[:], in_=pooled[:, half * 128:(half + 1) * 128],
                                    identity=ident_t[0:2, 0:2])
                nc.scalar.activation(out=pooledT[:, half, :], in_=tp[:], func=AF.Copy)
            o1ps = ps_mm.tile([2, HW1], F32, tag="mmps")
            for half in range(2):
                nc.tensor.matmul(out=o1ps[:], lhsT=pooledT[:, half, :],
                                 rhs=wh1_t[:, half, :], start=(half == 0),
                                 stop=(half == 1))
            o1 = work.tile([2, HW1], F16, tag="o1s")
            nc.vector.tensor_tensor(out=o1[:], in0=o1ps[:], in1=bh1_t[0:2, :], op=OP.add)
            nc.scalar.activation(out=o1[:], in_=o1[:], func=AF.Gelu)
            scr3 = work.tile([2, HW1], F16, tag="scr3")
            yv = small.tile([2, 1], F32, tag="yv")
            nc.vector.tensor_tensor(out=scr3[:], in0=o1[:], in1=wh2_t[0:2, :],
                                    op=OP.mult)
            nc.vector.tensor_reduce(out=yv[:], in_=scr3[:],
                                    axis=mybir.AxisListType.X, op=OP.add)
            nc.vector.tensor_scalar(out=yv[:], in0=yv[:], scalar1=bh2_t[0:2, :],
                                    scalar2=None, op0=OP.add)
            nc.sync.dma_start(out=y_d[:], in_=yv[:])

    nc.compile()
    return nc


# ----------------------------------------------------------------------------
# entry point
# ----------------------------------------------------------------------------
LAST_EXEC_NS = None
_LAST = {}


def rerun(n=3):
    """Re-execute the already-built program; returns min wall seconds."""
    import time
    from concourse.bass_utils import run_bass_kernel_spmd
    nc, meta = _LAST["nc"], _LAST["meta"]
    best = float("inf")
    for _ in range(n):
        t0 = time.time()
        run_bass_kernel_spmd(nc, meta["in_maps"], core_ids=list(range(NCORES)))
        best = min(best, time.time() - t0)
    return best


def kernel(**inputs):
    global LAST_EXEC_NS
    import os
    from concourse.bass_utils import run_bass_kernel_spmd
    from concourse.bass_interp import get_hw_module

    meta = prepare(inputs)
    nc = build(meta)
    nc.m = get_hw_module(nc.m)
    trace = bool(os.environ.get("GNN_TRACE"))
    res = run_bass_kernel_spmd(nc, meta["in_maps"], core_ids=list(range(NCORES)),
                               trace=trace)
    LAST_EXEC_NS = res.exec_time_ns
    _LAST.update(nc=nc, meta=meta)
    out = np.zeros(B, np.float32)
    for c in range(NCORES):
        yv = res.results[c]["y"].reshape(2)
        ga, gb = meta["glist"][c]
        out[ga] = yv[0]
        out[gb] = yv[1]
    return out



# revision 2
# speedup vs baseline: 41.6612x; 41.6612x over previous
"""Trainium2 Bass kernel for nn_BindingGNN (GATv2-style message-passing GNN).

Strategy (8 NeuronCores, SPMD, single NEFF):
  - Nodes partitioned by graph (2 graphs/core, edge-balanced pairing).
  - Edges owned by the core of their dst node, sorted by dst, packed into
    128-edge chunks grouped under 128-node-slot groups (uniform static
    structure across cores; per-core variation lives in input data only).
  - Per layer: xl table (all nodes) recomputed per-core from the AllGathered
    feature-major h; per-edge gather of xl rows via dma_gather; xr-expand and
    edge-feature projection via small selector matmuls (host-built 0/1
    blobs); attention softmax without max-subtraction (values are O(1),
    constant bias for range safety); segment-sum aggregation via selector
    matmuls accumulating in PSUM.
  - Cross-core comm: AllGather of the feature-major local h between layers.
  - Pooling/head computed fully locally (graphs are core-local).
Everything is fp16 on-chip with f32 PSUM/statistics.
"""
import sys
import numpy as np

sys.path.insert(0, "/opt/trn_rl_repo")

import concourse.bass as bass  # noqa: E402
import concourse.bacc as bacc  # noqa: E402
import concourse.tile as tile  # noqa: E402
from concourse import mybir  # noqa: E402
from concourse.masks import make_identity  # noqa: E402

F16 = mybir.dt.float16
F32 = mybir.dt.float32
I16 = mybir.dt.int16
AF = mybir.ActivationFunctionType
OP = mybir.AluOpType

HID = 256
NODE_DIM = 1280
L = 4
H = 4
DH = 64
EH = 16
B = 16
NCORES = 8
KX = 11  # ceil((1280+1)/128)
EXP_BIAS = -3.0
GATE_BIAS = -2.0


# ----------------------------------------------------------------------------
# host-side math (edge MLP is static per-edge preprocessing)
# ----------------------------------------------------------------------------
def _erf(x):
    # Abramowitz-Stegun is not accurate enough; use scipy if present else
    # a high-accuracy rational approx via numpy tanh-free formulation.
    try:
        from scipy.special import erf
        return erf(x)
    except Exception:
        import math
        v = np.vectorize(math.erf)
        return v(x).astype(x.dtype)


def _gelu_np(x):
    x64 = x.astype(np.float64)
    return (0.5 * x64 * (1.0 + _erf(x64 / np.sqrt(2.0)))).astype(np.float32)


def _edge_mlp_host(edge_attr, W_e1, b_e1, W_e2, b_e2):
    e = _gelu_np(edge_attr @ W_e1 + b_e1) @ W_e2 + b_e2
    return e.astype(np.float32)


# ----------------------------------------------------------------------------
# host-side sharding / blob construction
# ----------------------------------------------------------------------------
def prepare(inputs):
    x = np.asarray(inputs["x"], np.float32)
    edge_attr = np.asarray(inputs["edge_attr"], np.float32)
    edge_index = np.asarray(inputs["edge_index"]).astype(np.int64)
    batch = np.asarray(inputs["batch"]).astype(np.int64)
    N = x.shape[0]
    E = edge_index.shape[1]

    e_feat = _edge_mlp_host(edge_attr, np.asarray(inputs["W_e1"], np.float32),
                            np.asarray(inputs["b_e1"], np.float32),
                            np.asarray(inputs["W_e2"], np.float32),
                            np.asarray(inputs["b_e2"], np.float32))
    e_mean = e_feat.mean(0)

    # graph node ranges (batch is sorted)
    gcounts = np.bincount(batch, minlength=B)
    gstart = np.zeros(B + 1, np.int64)
    gstart[1:] = np.cumsum(gcounts)

    # weight per graph ~ edges into it + nodes (self-loops)
    dst_graph = batch[edge_index[1]]
    gedges = np.bincount(dst_graph, minlength=B) + gcounts
    order = np.argsort(-gedges, kind="stable")
    glist = [sorted([int(order[i]), int(order[B - 1 - i])]) for i in range(NCORES)]

    # per-core local node lists
    loc2glob = []
    for c in range(NCORES):
        ga, gb = glist[c]
        loc2glob.append(np.concatenate([np.arange(gstart[ga], gstart[ga + 1]),
                                        np.arange(gstart[gb], gstart[gb + 1])]))
    n_loc = np.array([len(v) for v in loc2glob])
    NLOC = int(-(-n_loc.max() // 128) * 128)
    NT = NLOC // 128
    GLOB = NCORES * NLOC
    assert GLOB < 32768, "padded node table must fit int16 indices"

    core_of = np.zeros(N, np.int64)
    slot_of = np.zeros(N, np.int64)
    for c in range(NCORES):
        core_of[loc2glob[c]] = c
        slot_of[loc2glob[c]] = np.arange(len(loc2glob[c]))
    padded_id = core_of * NLOC + slot_of  # global padded table row per node

    # ---- per-core edge lists (real edges + self-loops for all NLOC slots)
    core_edges = []  # (src_padded, dst_slot, efeat)
    for c in range(NCORES):
        sel = core_of[edge_index[1]] == c
        src_p = padded_id[edge_index[0][sel]]
        dst_s = slot_of[edge_index[1][sel]]
        ef = e_feat[sel]
        # self-loops for every slot (including padded slots: keeps den>0)
        sl_src = c * NLOC + np.arange(NLOC)
        sl_dst = np.arange(NLOC)
        sl_ef = np.broadcast_to(e_mean, (NLOC, EH))
        src_p = np.concatenate([src_p, sl_src])
        dst_s = np.concatenate([dst_s, sl_dst])
        ef = np.concatenate([ef, sl_ef], axis=0).astype(np.float32)
        o = np.argsort(dst_s, kind="stable")
        core_edges.append((src_p[o], dst_s[o], ef[o]))

    # ---- chunk structure: groups of 128 slots, CPG chunks per group
    CPG = 0
    for c in range(NCORES):
        dst_s = core_edges[c][1]
        gcnt = np.bincount(dst_s // 128, minlength=NT)
        CPG = max(CPG, int(-(-gcnt.max() // 128)))
    CPG += CPG % 2  # even, so chunk-pair DMA packing divides evenly
    NCH = NT * CPG
    SLOTS = NCH * 128

    per_core = []
    for c in range(NCORES):
        src_p, dst_s, ef = core_edges[c]
        M = len(src_p)
        grp = dst_s // 128
        # rank of edge within its group
        gcnt = np.bincount(grp, minlength=NT)
        goff = np.zeros(NT + 1, np.int64)
        goff[1:] = np.cumsum(gcnt)
        rank = np.arange(M) - goff[grp]
        pos = grp * (CPG * 128) + rank  # slot position in padded stream
        assert pos.max() < SLOTS

        srcs = np.zeros(SLOTS, np.int16)
        srcs[pos] = src_p.astype(np.int16)
        dsts = np.full(SLOTS, -1, np.int64)
        dsts[pos] = dst_s
        efs = np.zeros((SLOTS, EH), np.float32)
        efs[pos] = ef

        ch = np.arange(SLOTS) // 128
        ei = np.arange(SLOTS) % 128
        valid = dsts >= 0
        r = np.where(valid, dsts - (ch // CPG) * 128, 0)

        scb = np.zeros((NCH, 128, 128), np.float16)
        sctb = np.zeros((NCH, 128, 128), np.float16)
        scb[ch[valid], r[valid], ei[valid]] = 1.0
        sctb[ch[valid], ei[valid], r[valid]] = 1.0
        ecb = np.zeros((NCH, 17, 128), np.float16)
        ecb[:, :16, :] = efs.reshape(NCH, 128, EH).transpose(0, 2, 1).astype(np.float16)
        ecb[:, 16, :] = 1.0

        # pack pairs along free dim so per-partition DMA lines are 512B
        assert NCH % 2 == 0
        scb2 = scb.reshape(NCH // 2, 2, 128, 128).transpose(0, 2, 1, 3).reshape(NCH // 2, 128, 256)
        sctb2 = sctb.reshape(NCH // 2, 2, 128, 128).transpose(0, 2, 1, 3).reshape(NCH // 2, 128, 256)
        ecb2 = ecb.reshape(NCH // 2, 2, 17, 128).transpose(0, 2, 1, 3).reshape(NCH // 2, 17, 256)

        # dma_gather index layout: idx j at [j%16, j//16], replicated over 128 partitions
        idx16 = srcs.reshape(SLOTS // 16, 16).T  # [16, SLOTS//16]
        idx128 = np.tile(idx16, (8, 1)).astype(np.int16)

        # gmask: one-hot graph per slot
        gm = np.zeros((NLOC, 2), np.float16)
        ga, gb = glist[c]
        na = gstart[ga + 1] - gstart[ga]
        nb = gstart[gb + 1] - gstart[gb]
        gm[:na, 0] = 1.0
        gm[na:na + nb, 1] = 1.0
        gmask = gm.reshape(NT, 128, 2)

        # xT (padded transposed x for this core's slots)
        xT = np.zeros((KX * 128, NLOC), np.float16)
        xT[:NODE_DIM, :len(loc2glob[c])] = x[loc2glob[c]].T.astype(np.float16)
        xT[NODE_DIM, :] = 1.0  # bias row (applies +b_in; harmless for padding)

        per_core.append(dict(scb=scb2, sctb=sctb2, ecb=ecb2, idx=idx128,
                             gmask=gmask, xT=xT))
        per_core[-1]["_dbg"] = dict(srcs=srcs, dsts=dsts, efs=efs)

    # ---- shared weights
    f32 = np.float32
    W_in = np.asarray(inputs["W_in"], f32)
    b_in = np.asarray(inputs["b_in"], f32)
    winp = np.zeros((KX * 128, HID), f32)
    winp[:NODE_DIM] = W_in
    winp[NODE_DIM] = b_in
    winp = winp.reshape(KX, 128, HID).astype(np.float16)

    def rep(v):  # replicate a [HID] vector across partitions
        return np.broadcast_to(np.asarray(v, f32), (128, HID)).astype(np.float16).copy()

    Wl = np.asarray(inputs["Wl"], f32)
    Wr = np.asarray(inputs["Wr"], f32)
    bl = np.asarray(inputs["bl"], f32)
    br = np.asarray(inputs["br"], f32)
    We = np.asarray(inputs["We"], f32)
    att = np.asarray(inputs["att"], f32)
    bconv = np.asarray(inputs["bconv"], f32)
    ln_g = np.asarray(inputs["ln_g"], f32)
    ln_b = np.asarray(inputs["ln_b"], f32)

    wl = Wl.reshape(L, 2, 128, HID).astype(np.float16)
    wr = Wr.reshape(L, 2, 128, HID).astype(np.float16)
    weaug = np.zeros((L, 17, HID), f32)
    weaug[:, :16] = We
    weaug[:, 16] = bl + br
    weaug = weaug.astype(np.float16)
    attf = np.stack([rep(att[i].reshape(HID)) for i in range(L)])
    bconv2 = np.stack([rep(bconv[i] + bl[i]) for i in range(L)])
    lng = np.stack([rep(ln_g[i]) for i in range(L)])
    lnb = np.stack([rep(ln_b[i]) for i in range(L)])

    Wg1 = np.asarray(inputs["Wg1"], f32)  # (256,128)
    Wg2 = np.asarray(inputs["Wg2"], f32)  # (128,1)
    Wh1 = np.asarray(inputs["Wh1"], f32)  # (256,64)
    Wh2 = np.asarray(inputs["Wh2"], f32)  # (64,1)
    GW = Wg1.shape[1]
    HW1 = Wh1.shape[1]
    shared = dict(
        winp=winp,
        lnin_g=rep(inputs["ln_in_g"]), lnin_b=rep(inputs["ln_in_b"]),
        wl=wl, wr=wr, weaug=weaug, attf=attf, bconv2=bconv2, lng=lng, lnb=lnb,
        wg1=Wg1.reshape(2, 128, GW).astype(np.float16),
        bg1=np.asarray(inputs["bg1"], f32).reshape(1, GW).astype(np.float16),
        wg2=np.broadcast_to(Wg2.reshape(GW), (128, GW)).astype(np.float16).copy(),
        bg2=np.full((128, 1), float(np.asarray(inputs["bg2"]).reshape(())), f32),
        wh1=Wh1.reshape(2, 128, HW1).astype(np.float16),
        bh1=np.broadcast_to(np.asarray(inputs["bh1"], f32), (128, HW1)).astype(np.float16).copy(),
        wh2=np.broadcast_to(Wh2.reshape(HW1), (128, HW1)).astype(np.float16).copy(),
        bh2=np.full((128, 1), float(np.asarray(inputs["bh2"]).reshape(())), f32),
    )

    in_maps = []
    dbg = []
    for c in range(NCORES):
        m = dict(shared)
        m.update(per_core[c])
        dbg.append(m.pop("_dbg", None))
        in_maps.append({k: np.ascontiguousarray(v) for k, v in m.items()})

    meta = dict(NLOC=NLOC, NT=NT, CPG=CPG, NCH=NCH, SLOTS=SLOTS, GLOB=GLOB,
                glist=glist, GW=GW, HW1=HW1, in_maps=in_maps, dbg=dbg,
                loc2glob=loc2glob)
    return meta


# ----------------------------------------------------------------------------
# device program
# ----------------------------------------------------------------------------
def build(meta, num_devices=NCORES, nlayers=L):
    NLOC, NT, CPG, NCH = meta["NLOC"], meta["NT"], meta["CPG"], meta["NCH"]
    SLOTS, GW, HW1 = meta["SLOTS"], meta["GW"], meta["HW1"]
    ICOLS = SLOTS // 16
    SUP = 8  # chunks per supergather (dma_gather fails above 1024 idxs/call)
    NSUP = -(-NCH // SUP)

    nc = bacc.Bacc("TRN2", target_bir_lowering=False, debug=False,
                   enable_asserts=True, num_devices=num_devices)

    def din(name, shape, dt=F16):
        return nc.dram_tensor(name, list(shape), dt, kind="ExternalInput").ap()

    # inputs
    xT_d = din("xT", (KX * 128, NLOC))
    winp_d = din("winp", (KX, 128, HID))
    lnin_g_d = din("lnin_g", (128, HID))
    lnin_b_d = din("lnin_b", (128, HID))
    wl_d = din("wl", (L, 2, 128, HID))
    wr_d = din("wr", (L, 2, 128, HID))
    weaug_d = din("weaug", (L, 17, HID))
    attf_d = din("attf", (L, 128, HID))
    bconv2_d = din("bconv2", (L, 128, HID))
    lng_d = din("lng", (L, 128, HID))
    lnb_d = din("lnb", (L, 128, HID))
    scb_d = din("scb", (NCH // 2, 128, 256))
    sctb_d = din("sctb", (NCH // 2, 128, 256))
    ecb_d = din("ecb", (NCH // 2, 17, 256))
    idx_d = din("idx", (128, ICOLS), I16)
    gmask_d = din("gmask", (NT, 128, 2))
    wg1_d = din("wg1", (2, 128, GW))
    bg1_d = din("bg1", (1, GW))
    wg2_d = din("wg2", (128, GW))
    bg2_d = din("bg2", (128, 1), F32)
    wh1_d = din("wh1", (2, 128, HW1))
    bh1_d = din("bh1", (128, HW1))
    wh2_d = din("wh2", (128, HW1))
    bh2_d = din("bh2", (128, 1), F32)
    y_d = nc.dram_tensor("y", [2, 1], F32, kind="ExternalOutput").ap()

    # internal DRAM
    hloc_d = nc.dram_tensor("hloc", [NLOC, HID], F16).ap()
    bounce_d = nc.dram_tensor("bounce", [2, 128, NLOC], F16).ap()
    xl_d = nc.dram_tensor("xl_table", [NCORES * NLOC, HID], F16).ap()
    hTg_d = [nc.dram_tensor(f"hTg{i}", [NCORES, 2, 128, NLOC], F16,
                            addr_space="Shared").ap() for i in range(L)]

    rg = [list(range(num_devices))]

    with tile.TileContext(nc) as tc:
        import contextlib
        ctx = contextlib.ExitStack()
        with ctx:
            const = ctx.enter_context(tc.tile_pool(name="const", bufs=1))
            work = ctx.enter_context(tc.tile_pool(name="work", bufs=3))
            small = ctx.enter_context(tc.tile_pool(name="small", bufs=4))
            xtp = ctx.enter_context(tc.tile_pool(name="xtp", bufs=3))
            scp = ctx.enter_context(tc.tile_pool(name="scp", bufs=3))
            xlg_p = ctx.enter_context(tc.tile_pool(name="xlg", bufs=2))
            ps_mm = ctx.enter_context(tc.tile_pool(name="ps_mm", bufs=2, space="PSUM"))
            ps_ed = ctx.enter_context(tc.tile_pool(name="ps_ed", bufs=2, space="PSUM"))
            ps_ag = ctx.enter_context(tc.tile_pool(name="ps_ag", bufs=2, space="PSUM"))

            # ---------------- resident tiles
            winp_t = const.tile([128, KX, HID], F16)
            nc.sync.dma_start(out=winp_t[:], in_=winp_d.rearrange("k p f -> p k f"))
            lnin_g_t = const.tile([128, HID], F16)
            nc.sync.dma_start(out=lnin_g_t[:], in_=lnin_g_d[:])
            lnin_b_t = const.tile([128, HID], F16)
            nc.sync.dma_start(out=lnin_b_t[:], in_=lnin_b_d[:])
            wl_t = const.tile([128, L, 2, HID], F16)
            nc.sync.dma_start(out=wl_t[:], in_=wl_d.rearrange("l k p f -> p l k f"))
            wr_t = const.tile([128, L, 2, HID], F16)
            nc.sync.dma_start(out=wr_t[:], in_=wr_d.rearrange("l k p f -> p l k f"))
            weaug_t = const.tile([17, L, HID], F16)
            nc.sync.dma_start(out=weaug_t[:], in_=weaug_d.rearrange("l p f -> p l f"))
            attf_t = const.tile([128, L, HID], F16)
            nc.sync.dma_start(out=attf_t[:], in_=attf_d.rearrange("l p f -> p l f"))
            bconv2_t = const.tile([128, L, HID], F16)
            nc.sync.dma_start(out=bconv2_t[:], in_=bconv2_d.rearrange("l p f -> p l f"))
            lng_t = const.tile([128, L, HID], F16)
            nc.sync.dma_start(out=lng_t[:], in_=lng_d.rearrange("l p f -> p l f"))
            lnb_t = const.tile([128, L, HID], F16)
            nc.sync.dma_start(out=lnb_t[:], in_=lnb_d.rearrange("l p f -> p l f"))
            idx_t = const.tile([128, ICOLS], I16)
            nc.sync.dma_start(out=idx_t[:], in_=idx_d[:])
            gmask_t = const.tile([128, NT, 2], F16)
            nc.sync.dma_start(out=gmask_t[:], in_=gmask_d.rearrange("t p g -> p t g"))
            wg1_t = const.tile([128, 2, GW], F16)
            nc.sync.dma_start(out=wg1_t[:], in_=wg1_d.rearrange("k p f -> p k f"))
            bg1_t = const.tile([1, GW], F16)
            nc.sync.dma_start(out=bg1_t[:], in_=bg1_d[:])
            wg2_t = const.tile([128, GW], F16)
            nc.sync.dma_start(out=wg2_t[:], in_=wg2_d[:])
            bg2_t = const.tile([128, 1], F32)
            nc.sync.dma_start(out=bg2_t[:], in_=bg2_d[:])
            wh1_t = const.tile([128, 2, HW1], F16)
            nc.sync.dma_start(out=wh1_t[:], in_=wh1_d.rearrange("k p f -> p k f"))
            bh1_t = const.tile([128, HW1], F16)
            nc.sync.dma_start(out=bh1_t[:], in_=bh1_d[:])
            wh2_t = const.tile([128, HW1], F16)
            nc.sync.dma_start(out=wh2_t[:], in_=wh2_d[:])
            bh2_t = const.tile([128, 1], F32)
            nc.sync.dma_start(out=bh2_t[:], in_=bh2_d[:])

            h_res = const.tile([128, NT, HID + 1], F16)
            hT_loc = const.tile([128, 2, NLOC], F16)
            xr_t = const.tile([128, NT, HID], F16)
            ones1_t = const.tile([1, 128], F16)
            nc.vector.memset(ones1_t[:], 1.0)
            eps_t = const.tile([128, 1], F32)
            nc.vector.memset(eps_t[:], 1e-5)
            expb_t = const.tile([128, 1], F32)
            nc.vector.memset(expb_t[:], EXP_BIAS)
            gateb_t = const.tile([128, 1], F32)
            nc.vector.memset(gateb_t[:], GATE_BIAS)
            ident_t = const.tile([128, 128], F16)
            make_identity(nc, ident_t[:])
            for t in range(NT):
                nc.vector.memset(h_res[:, t, HID:HID + 1], 1.0)

            def refine_recip(r_ap, x_ap, shape, tag):
                # r <- r*(2 - x*r), one Newton step on a LUT seed
                t = small.tile(shape, F32, tag=tag)
                nc.vector.tensor_tensor(out=t[:], in0=x_ap, in1=r_ap, op=OP.mult)
                nc.vector.tensor_scalar(out=t[:], in0=t[:], scalar1=2.0,
                                        scalar2=-1.0, op0=OP.subtract, op1=OP.mult)
                nc.vector.tensor_tensor(out=r_ap, in0=r_ap, in1=t[:], op=OP.mult)

            def refine_rsqrt(r_ap, x_ap, shape, tag):
                # r <- 0.5*r*(3 - x*r*r)
                t = small.tile(shape, F32, tag=tag)
                nc.vector.tensor_tensor(out=t[:], in0=r_ap, in1=r_ap, op=OP.mult)
                nc.vector.tensor_tensor(out=t[:], in0=x_ap, in1=t[:], op=OP.mult)
                nc.vector.tensor_scalar(out=t[:], in0=t[:], scalar1=3.0,
                                        scalar2=-0.5, op0=OP.subtract, op1=OP.mult)
                nc.vector.tensor_tensor(out=r_ap, in0=r_ap, in1=t[:], op=OP.mult)

            # ---------------- LN helper: s_t fp16 [128,HID] + musum f32 -> dest
            def layernorm(s_t, musum, g_ap, b_ap, dest_ap, gelu_after=False):
                mu = small.tile([128, 1], F32, tag="mu")
                nc.vector.tensor_scalar(out=mu[:], in0=musum, scalar1=1.0 / HID,
                                        scalar2=None, op0=OP.mult)
                d_t = work.tile([128, HID], F16, tag="d")
                nc.vector.tensor_scalar(out=d_t[:], in0=s_t, scalar1=mu[:],
                                        scalar2=None, op0=OP.subtract)
                scr = work.tile([128, HID], F16, tag="scr")
                vs = small.tile([128, 1], F32, tag="vs")
                nc.vector.tensor_tensor(out=scr[:], in0=d_t[:], in1=d_t[:], op=OP.mult)
                nc.vector.tensor_reduce(out=vs[:], in_=scr[:],
                                        axis=mybir.AxisListType.X, op=OP.add)
                vx = small.tile([128, 1], F32, tag="vx")
                nc.vector.tensor_scalar(out=vx[:], in0=vs[:], scalar1=1.0 / HID,
                                        scalar2=None, op0=OP.mult)
                nc.vector.tensor_scalar(out=vx[:], in0=vx[:], scalar1=eps_t[:],
                                        scalar2=None, op0=OP.add)
                sd = small.tile([128, 1], F32, tag="sd")
                nc.scalar.activation(out=sd[:], in_=vx[:], func=AF.Ln)
                rstd = small.tile([128, 1], F32, tag="rstd")
                nc.scalar.activation(out=rstd[:], in_=sd[:], func=AF.Exp, scale=-0.5)
                refine_rsqrt(rstd[:], vx[:], [128, 1], "nsr")
                n_t = work.tile([128, HID], F16, tag="n")
                nc.vector.tensor_scalar(out=n_t[:], in0=d_t[:], scalar1=rstd[:],
                                        scalar2=None, op0=OP.mult)
                nc.vector.tensor_tensor(out=n_t[:], in0=n_t[:], in1=g_ap, op=OP.mult)
                if gelu_after:
                    nc.vector.tensor_tensor(out=n_t[:], in0=n_t[:], in1=b_ap, op=OP.add)
                    nc.scalar.activation(out=dest_ap, in_=n_t[:], func=AF.Gelu)
                else:
                    nc.vector.tensor_tensor(out=dest_ap, in0=n_t[:], in1=b_ap, op=OP.add)

            # ---------------- phase A: input projection (local nodes)
            for t2 in range((NT + 1) // 2):
                tcnt = min(2, NT - t2 * 2)
                xt_t = xtp.tile([128, KX, 2 * 128], F16, tag="xt")
                for k in range(KX):
                    nc.sync.dma_start(
                        out=xt_t[:, k, :tcnt * 128],
                        in_=xT_d[k * 128:(k + 1) * 128,
                                 t2 * 256:t2 * 256 + tcnt * 128])
                for j in range(tcnt):
                    t = t2 * 2 + j
                    ps = ps_mm.tile([128, HID], F32, tag="mmps")
                    for k in range(KX):
                        nc.tensor.matmul(out=ps[:], lhsT=xt_t[:, k, j * 128:(j + 1) * 128],
                                         rhs=winp_t[:, k, :], start=(k == 0),
                                         stop=(k == KX - 1))
                    s_t = work.tile([128, HID], F16, tag="s")
                    musum = small.tile([128, 1], F32, tag="musum")
                    nc.scalar.activation(out=s_t[:], in_=ps[:], func=AF.Copy,
                                         accum_out=musum[:])
                    layernorm(s_t[:], musum[:], lnin_g_t[:], lnin_b_t[:],
                              h_res[:, t, :HID], gelu_after=True)
                    nc.sync.dma_start(out=hloc_d[t * 128:(t + 1) * 128, :],
                                      in_=h_res[:, t, :HID])

            # ---------------- per layer
            def transpose_and_gather(layer):
                # hloc (node-major, HBM) -> hT_loc (feature-major, SBUF)
                for half in range(2):
                    nc.sync.dma_start(out=hT_loc[:, half, :],
                                      in_=hloc_d[:, half * 128:(half + 1) * 128],
                                      transpose=True)
                if layer < L:
                    nc.sync.dma_start(out=bounce_d.rearrange("h p n -> p h n"),
                                      in_=hT_loc[:])
                    nc.gpsimd.collective_compute(
                        "AllGather", OP.bypass, replica_groups=rg,
                        ins=[bounce_d[:]], outs=[hTg_d[layer][:]])

            transpose_and_gather(0)

            for i in range(nlayers):
                # xl table over all ranks; xr for local nodes
                for r in range(NCORES):
                    for t2 in range((NT + 1) // 2):
                        tcnt = min(2, NT - t2 * 2)
                        ht = xtp.tile([128, 2, 2 * 128], F16, tag="ht")
                        for half in range(2):
                            nc.sync.dma_start(
                                out=ht[:, half, :tcnt * 128],
                                in_=hTg_d[i][r, half, :,
                                             t2 * 256:t2 * 256 + tcnt * 128])
                        for j in range(tcnt):
                            t = t2 * 2 + j
                            ps = ps_mm.tile([128, HID], F32, tag="mmps")
                            for half in range(2):
                                nc.tensor.matmul(
                                    out=ps[:], lhsT=ht[:, half, j * 128:(j + 1) * 128],
                                    rhs=wl_t[:, i, half, :],
                                    start=(half == 0), stop=(half == 1))
                            xl_t = work.tile([128, HID], F16, tag="xlt")
                            nc.scalar.activation(out=xl_t[:], in_=ps[:], func=AF.Copy)
                            row = (r * NT + t) * 128
                            nc.sync.dma_start(out=xl_d[row:row + 128, :], in_=xl_t[:])
                for t in range(NT):
                    ps = ps_mm.tile([128, HID], F32, tag="mmps")
                    for half in range(2):
                        nc.tensor.matmul(out=ps[:],
                                         lhsT=hT_loc[:, half, t * 128:(t + 1) * 128],
                                         rhs=wr_t[:, i, half, :],
                                         start=(half == 0), stop=(half == 1))
                    nc.scalar.activation(out=xr_t[:, t, :], in_=ps[:], func=AF.Copy)

                # edge phase
                xlg_tiles = {}
                agg = None
                for chk in range(NCH):
                    s, joff = divmod(chk, SUP)
                    if joff == 0:
                        cnt = min(SUP, NCH - s * SUP)
                        xlg = xlg_p.tile([128, SUP, HID], F16, tag="xlg")
                        nc.gpsimd.dma_gather(
                            out_ap=xlg[:, :cnt, :], in_ap=xl_d[:, :],
                            idxs_ap=idx_t[:, s * (SUP * 8):s * (SUP * 8) + cnt * 8],
                            num_idxs=cnt * 128, num_idxs_reg=cnt * 128,
                            elem_size=HID)
                        xlg_tiles[s] = xlg
                    xlg = xlg_tiles[s]
                    g, cidx = divmod(chk, CPG)

                    if chk % 2 == 0:
                        sc2 = scp.tile([128, 256], F16, tag="sc2")
                        nc.sync.dma_start(out=sc2[:], in_=scb_d[chk // 2])
                        sct2 = scp.tile([128, 256], F16, tag="sct2")
                        nc.sync.dma_start(out=sct2[:], in_=sctb_d[chk // 2])
                        ec2 = scp.tile([17, 256], F16, tag="ec2")
                        nc.sync.dma_start(out=ec2[:], in_=ecb_d[chk // 2])
                        sc2_cur, sct2_cur, ec2_cur = sc2, sct2, ec2
                    half = (chk % 2) * 128

                    ps = ps_ed.tile([128, HID], F32, tag="edps")
                    nc.tensor.matmul(out=ps[:], lhsT=sc2_cur[:, half:half + 128],
                                     rhs=xr_t[:, g, :], start=True, stop=False)
                    nc.tensor.matmul(out=ps[:], lhsT=ec2_cur[:, half:half + 128],
                                     rhs=weaug_t[:, i, :], start=False, stop=True)
                    m_t = work.tile([128, HID], F16, tag="m")
                    nc.vector.tensor_tensor(out=m_t[:], in0=xlg[:, joff, :],
                                            in1=ps[:], op=OP.add)
                    lr_t = work.tile([128, HID], F16, tag="lr")
                    nc.scalar.activation(out=lr_t[:], in_=m_t[:], func=AF.Copy,
                                         scale=0.2)
                    nc.vector.tensor_tensor(out=m_t[:], in0=m_t[:], in1=lr_t[:],
                                            op=OP.max)
                    v_t = work.tile([128, HID], F16, tag="v")
                    nc.vector.tensor_tensor(out=v_t[:], in0=m_t[:],
                                            in1=attf_t[:, i, :], op=OP.mult)
                    a_t = small.tile([128, H], F32, tag="a")
                    nc.vector.tensor_reduce(
                        out=a_t[:], in_=v_t[:].rearrange("p (h d) -> p h d", d=DH),
                        axis=mybir.AxisListType.X, op=OP.add)
                    u_t = work.tile([128, HID + H], F16, tag="u")
                    nc.scalar.activation(out=u_t[:, HID:HID + H], in_=a_t[:],
                                         func=AF.Exp, bias=expb_t[:])
                    nc.vector.tensor_tensor(
                        out=u_t[:, :HID].rearrange("p (h d) -> p h d", d=DH),
                        in0=xlg[:, joff, :].rearrange("p (h d) -> p h d", d=DH),
                        in1=u_t[:, HID:HID + H].to_broadcast([128, H, DH]),
                        op=OP.mult)
                    if cidx == 0:
                        agg = ps_ag.tile([128, HID + H], F32, tag="agg")
                    nc.tensor.matmul(out=agg[:], lhsT=sct2_cur[:, half:half + 128],
                                     rhs=u_t[:], start=(cidx == 0),
                                     stop=(cidx == CPG - 1))

                    if cidx == CPG - 1:
                        rd = small.tile([128, H], F32, tag="rd")
                        nc.scalar.activation(out=rd[:], in_=agg[:, HID:HID + H],
                                             func=AF.Ln)
                        nc.scalar.activation(out=rd[:], in_=rd[:], func=AF.Exp,
                                             scale=-1.0)
                        refine_recip(rd[:], agg[:, HID:HID + H], [128, H], "nrd")
                        o_t = work.tile([128, HID], F16, tag="o")
                        nc.vector.tensor_tensor(
                            out=o_t[:].rearrange("p (h d) -> p h d", d=DH),
                            in0=agg[:, :HID].rearrange("p (h d) -> p h d", d=DH),
                            in1=rd[:].to_broadcast([128, H, DH]), op=OP.mult)
                        nc.vector.tensor_tensor(out=o_t[:], in0=o_t[:],
                                                in1=bconv2_t[:, i, :], op=OP.add)
                        nc.scalar.activation(out=o_t[:], in_=o_t[:], func=AF.Gelu)
                        s_t = work.tile([128, HID], F16, tag="s")
                        musum = small.tile([128, 1], F32, tag="musum")
                        nc.vector.tensor_tensor(out=s_t[:], in0=o_t[:],
                                                in1=h_res[:, g, :HID], op=OP.add)
                        nc.vector.tensor_reduce(out=musum[:], in_=s_t[:],
                                                axis=mybir.AxisListType.X, op=OP.add)
                        layernorm(s_t[:], musum[:], lng_t[:, i, :], lnb_t[:, i, :],
                                  h_res[:, g, :HID])
                        nc.sync.dma_start(out=hloc_d[g * 128:(g + 1) * 128, :],
                                          in_=h_res[:, g, :HID])
                transpose_and_gather(i + 1)

            # ---------------- pooling + head
            pool_ps = ps_mm.tile([2, HID + 1], F32, tag="mmps")
            for t in range(NT):
                g1 = ps_mm.tile([128, GW], F32, tag="mmps")
                for half in range(2):
                    nc.tensor.matmul(out=g1[:],
                                     lhsT=hT_loc[:, half, t * 128:(t + 1) * 128],
                                     rhs=wg1_t[:, half, :], start=(half == 0),
                                     stop=False)
                nc.tensor.matmul(out=g1[:], lhsT=ones1_t[:],
                                 rhs=bg1_t[:], start=False, stop=True)
                t_t = work.tile([128, GW], F16, tag="tt")
                nc.scalar.activation(out=t_t[:], in_=g1[:], func=AF.Tanh)
                scr = work.tile([128, GW], F16, tag="scr2")
                gate = small.tile([128, 1], F32, tag="gate")
                nc.vector.tensor_tensor(out=scr[:], in0=t_t[:], in1=wg2_t[:],
                                        op=OP.mult)
                nc.vector.tensor_reduce(out=gate[:], in_=scr[:],
                                        axis=mybir.AxisListType.X, op=OP.add)
                nc.vector.tensor_scalar(out=gate[:], in0=gate[:], scalar1=bg2_t[:],
                                        scalar2=None, op0=OP.add)
                eg = small.tile([128, 1], F16, tag="eg")
                nc.scalar.activation(out=eg[:], in_=gate[:], func=AF.Exp,
                                     bias=gateb_t[:])
                wm = small.tile([128, 2], F16, tag="wm")
                nc.vector.tensor_tensor(out=wm[:], in0=gmask_t[:, t, :],
                                        in1=eg[:].to_broadcast([128, 2]), op=OP.mult)
                nc.tensor.matmul(out=pool_ps[:], lhsT=wm[:], rhs=h_res[:, t, :],
                                 start=(t == 0), stop=(t == NT - 1))
            rd = small.tile([2, 1], F32, tag="prd")
            nc.scalar.activation(out=rd[:], in_=pool_ps[:, HID:HID + 1], func=AF.Ln)
            nc.scalar.activation(out=rd[:], in_=rd[:], func=AF.Exp, scale=-1.0)
            refine_recip(rd[:], pool_ps[:, HID:HID + 1], [2, 1], "npd")
            pooled = work.tile([2, HID], F16, tag="pooled")
            nc.vector.tensor_scalar(out=pooled[:], in0=pool_ps[:, :HID],
                                    scalar1=rd[:], scalar2=None, op0=OP.mult)
            pooledT = work.tile([128, 2, 2], F16, tag="pooledT")
            for half in range(2):
                tp = ps_mm.tile([128, 2], F16, tag="mmps")
                nc.tensor.transpose(out=tp[:], in_=pooled[:, half * 128:(half + 1) * 128],
                                    identity=ident_t[0:2, 0:2])
                nc.scalar.activation(out=pooledT[:, half, :], in_=tp[:], func=AF.Copy)
            o1ps = ps_mm.tile([2, HW1], F32, tag="mmps")
            for half in range(2):
                nc.tensor.matmul(out=o1ps[:], lhsT=pooledT[:, half, :],
                                 rhs=wh1_t[:, half, :], start=(half == 0),
                                 stop=(half == 1))
            o1 = work.tile([2, HW1], F16, tag="o1s")
            nc.vector.tensor_tensor(out=o1[:], in0=o1ps[:], in1=bh1_t[0:2, :], op=OP.add)
            nc.scalar.activation(out=o1[:], in_=o1[:], func=AF.Gelu)
            scr3 = work.tile([2, HW1], F16, tag="scr3")
            yv = small.tile([2, 1], F32, tag="yv")
            nc.vector.tensor_tensor(out=scr3[:], in0=o1[:], in1=wh2_t[0:2, :],
                                    op=OP.mult)
            nc.vector.tensor_reduce(out=yv[:], in_=scr3[:],
                                    axis=mybir.AxisListType.X, op=OP.add)
            nc.vector.tensor_scalar(out=yv[:], in0=yv[:], scalar1=bh2_t[0:2, :],
                                    scalar2=None, op0=OP.add)
            nc.sync.dma_start(out=y_d[:], in_=yv[:])

    nc.compile()
    return nc


# ----------------------------------------------------------------------------
# persistent execution layer: compile once, keep inputs device-resident, so
# repeat executions measure kernel time rather than PJRT re-trace + re-stage.
# Mirrors concourse.bass2jax.run_bass_via_pjrt's lowering contract exactly.
# ----------------------------------------------------------------------------
def _make_runner(nc, in_maps, n_cores):
    import jax
    from jax.experimental.shard_map import shard_map
    from jax.sharding import Mesh, PartitionSpec, NamedSharding
    from concourse import bass2jax

    bass2jax.install_neuronx_cc_hook()

    if nc.dbg_addr is not None:
        if nc.dbg_callbacks:
            raise RuntimeError("dbg callbacks unsupported in persistent runner")
        in_maps = [{**m, nc.dbg_addr.name: np.zeros((1, 2), np.uint32)}
                   for m in in_maps]

    partition_name = nc.partition_id_tensor.name if nc.partition_id_tensor else None
    in_names, out_names, out_avals, zero_outs = [], [], [], []
    for alloc in nc.m.functions[0].allocations:
        if not isinstance(alloc, mybir.MemoryLocationSet):
            continue
        name = alloc.memorylocations[0].name
        if alloc.kind == "ExternalInput":
            if name != partition_name:
                in_names.append(name)
        elif alloc.kind == "ExternalOutput":
            shape = tuple(alloc.tensor_shape)
            dtype = mybir.dt.np(alloc.dtype)
            out_names.append(name)
            out_avals.append(jax.core.ShapedArray(shape, dtype))
            zero_outs.append(np.zeros(shape, dtype))
    n_params = len(in_names)
    n_outs = len(out_avals)
    in_names_all = list(in_names) + out_names
    if partition_name is not None:
        in_names_all.append(partition_name)
    donate = tuple(range(n_params, n_params + n_outs))

    def _body(*args):
        operands = list(args)
        if partition_name is not None:
            operands.append(bass2jax.partition_id_tensor())
        outs = bass2jax._bass_exec_p.bind(
            *operands,
            out_avals=tuple(out_avals),
            in_names=tuple(in_names_all),
            out_names=tuple(out_names),
            lowering_input_output_aliases=(),
            sim_require_finite=True,
            sim_require_nnan=True,
            nc=nc,
        )
        return tuple(outs)

    devices = jax.devices()[:n_cores]
    mesh = Mesh(np.asarray(devices), ("core",))
    in_specs = (PartitionSpec("core"),) * (n_params + n_outs)
    out_specs = (PartitionSpec("core"),) * n_outs
    fn = shard_map(_body, mesh=mesh, in_specs=in_specs, out_specs=out_specs,
                   check_rep=False)

    per_core = [[np.asarray(m[name]) for name in in_names] for m in in_maps]
    concat_in = [np.concatenate([per_core[c][i] for c in range(n_cores)], axis=0)
                 for i in range(n_params)]
    sh = NamedSharding(mesh, PartitionSpec("core"))
    dev_in = [jax.device_put(a, sh) for a in concat_in]
    zshapes = [(n_cores * z.shape[0], *z.shape[1:]) for z in zero_outs]
    zdtypes = [z.dtype for z in zero_outs]
    zavals = [jax.ShapeDtypeStruct(s, d, sharding=sh)
              for s, d in zip(zshapes, zdtypes)]

    compiled = bass2jax.fast_dispatch_compile(
        lambda: jax.jit(fn, donate_argnums=donate, keep_unused=True)
        .lower(*dev_in, *zavals).compile())

    def run():
        zeros = [jax.device_put(np.zeros(s, d), sh)
                 for s, d in zip(zshapes, zdtypes)]
        outs = jax.block_until_ready(compiled(*dev_in, *zeros))
        return [
            {name: np.asarray(outs[i]).reshape(n_cores, *out_avals[i].shape)[c]
             for i, name in enumerate(out_names)}
            for c in range(n_cores)
        ]
    return run


# ----------------------------------------------------------------------------
# entry point
# ----------------------------------------------------------------------------
LAST_EXEC_NS = None
_LAST = {}


def rerun(n=3):
    """Re-execute the already-built program; returns min wall seconds."""
    import time
    run = _LAST["run"]
    best = float("inf")
    for _ in range(n):
        t0 = time.time()
        run()
        best = min(best, time.time() - t0)
    return best


def kernel(**inputs):
    global LAST_EXEC_NS
    from concourse.bass_interp import get_hw_module

    meta = prepare(inputs)
    nc = build(meta)
    nc.m = get_hw_module(nc.m)
    run = _make_runner(nc, meta["in_maps"], NCORES)
    results = run()
    _LAST.update(nc=nc, meta=meta, run=run)
    out = np.zeros(B, np.float32)
    for c in range(NCORES):
        yv = results[c]["y"].reshape(2)
        ga, gb = meta["glist"][c]
        out[ga] = yv[0]
        out[gb] = yv[1]
    return out



# revision 20
# speedup vs baseline: 833.1464x; 19.9981x over previous
"""Trainium2 Bass kernel for nn_BindingGNN (GATv2-style message-passing GNN).

Strategy (8 NeuronCores, SPMD, single NEFF):
  - Nodes assigned freely to 8 cores x NT groups of 128 slots by LPT
    bin-packing on (indegree+1), so every group owns ~equal edge work and
    CPG (chunks per group) is minimal and uniform.
  - Edges owned by the core of their dst node, grouped under the dst's
    128-slot group, sorted by src within each group (gather locality),
    packed into 128-edge chunks (CPG chunks per group).
  - Per layer: xl computed locally (NT tiles), AllGathered node-major into
    a shared [NCORES*NLOC, HID] table; per-edge gather of xl rows via
    dma_gather; xr-expand / segment-sum selectors are 0/1 one-hots built
    ON DEVICE per chunk from tiny fp16 index rows (partition_broadcast +
    is_equal on the Pool engine) instead of 64KB/chunk HBM blobs;
    edge-feature projection via small matmul from a packed [17,256] blob;
    attention softmax without max-subtraction (constant bias for range
    safety); segment-sum aggregation via selector matmuls into PSUM.
  - Pooling: per-core partial attention-pool accumulated transposed
    ([feat, graph]) via matmuls, AllReduced ([128,48] f32), head computed
    redundantly on every core; host reads core 0.
Everything is fp16 on-chip with f32 PSUM/statistics.
"""
import sys
import numpy as np

sys.path.insert(0, "/opt/trn_rl_repo")

import concourse.bass as bass  # noqa: E402
import concourse.bacc as bacc  # noqa: E402
import concourse.tile as tile  # noqa: E402
from concourse import mybir  # noqa: E402
from concourse.masks import make_identity  # noqa: E402

F16 = mybir.dt.float16
F32 = mybir.dt.float32
I16 = mybir.dt.int16
AF = mybir.ActivationFunctionType
OP = mybir.AluOpType

HID = 256
NODE_DIM = 1280
L = 4
H = 4
DH = 64
EH = 16
B = 16
NCORES = 8
KX = 11  # ceil((1280+1)/128)
EXP_BIAS = -3.0
GATE_BIAS = -2.0


# ----------------------------------------------------------------------------
# host-side math (edge MLP is static per-edge preprocessing)
# ----------------------------------------------------------------------------
def _erf(x):
    try:
        from scipy.special import erf
        return erf(x)
    except Exception:
        import math
        v = np.vectorize(math.erf)
        return v(x).astype(x.dtype)


def _gelu_np(x):
    x64 = x.astype(np.float64)
    return (0.5 * x64 * (1.0 + _erf(x64 / np.sqrt(2.0)))).astype(np.float32)


def _edge_mlp_host(edge_attr, W_e1, b_e1, W_e2, b_e2):
    e = _gelu_np(edge_attr @ W_e1 + b_e1) @ W_e2 + b_e2
    return e.astype(np.float32)


# ----------------------------------------------------------------------------
# host-side sharding / blob construction
# ----------------------------------------------------------------------------
def prepare(inputs):
    import heapq
    x = np.asarray(inputs["x"], np.float32)
    edge_index = np.asarray(inputs["edge_index"]).astype(np.int64)
    batch = np.asarray(inputs["batch"]).astype(np.int64)
    N = x.shape[0]

    e_feat = _edge_mlp_host(np.asarray(inputs["edge_attr"], np.float32),
                            np.asarray(inputs["W_e1"], np.float32),
                            np.asarray(inputs["b_e1"], np.float32),
                            np.asarray(inputs["W_e2"], np.float32),
                            np.asarray(inputs["b_e2"], np.float32))
    e_mean = e_feat.mean(0)

    # ---- balanced node -> (core, group, slot) assignment (LPT on indeg+1)
    NT = -(-N // (NCORES * 128))
    NLOC = NT * 128
    NGRP = NCORES * NT
    GLOB = NCORES * NLOC
    assert GLOB < 32768, "padded node table must fit int16 indices"

    indeg = np.bincount(edge_index[1], minlength=N)
    w = indeg + 1
    heap = [(0, g) for g in range(NGRP)]
    heapq.heapify(heap)
    counts = np.zeros(NGRP, np.int64)
    group_of = np.empty(N, np.int64)
    slot_of = np.empty(N, np.int64)
    for n in np.argsort(-w, kind="stable"):
        while True:
            load, g = heapq.heappop(heap)
            if counts[g] < 128:
                break
        group_of[n] = g
        slot_of[n] = counts[g]
        counts[g] += 1
        heapq.heappush(heap, (load + int(w[n]), g))
    loads = np.zeros(NGRP, np.int64)
    np.add.at(loads, group_of, w)
    gtot = loads + (128 - counts)  # pad slots contribute one self-loop each
    CPG = int(-(-gtot.max() // 128))
    NCH = NT * CPG
    SLOTS = NCH * 128

    core_of_node = group_of // NT
    grp_in_core = group_of % NT
    padded_id = core_of_node * NLOC + grp_in_core * 128 + slot_of

    src_all = padded_id[edge_index[0]]
    dst_core = core_of_node[edge_index[1]]
    dst_grp = grp_in_core[edge_index[1]]
    dst_r = slot_of[edge_index[1]]

    NCHE = NCH + (NCH % 2)
    per_core = []
    dbg = []
    for c in range(NCORES):
        sel = dst_core == c
        sp = src_all[sel]
        gr = dst_grp[sel]
        rr = dst_r[sel]
        ef = e_feat[sel]
        # self-loops for every slot (including padded slots: keeps den>0)
        sp = np.concatenate([sp, c * NLOC + np.arange(NLOC)])
        gr = np.concatenate([gr, np.arange(NLOC) // 128])
        rr = np.concatenate([rr, np.arange(NLOC) % 128])
        ef = np.concatenate([ef, np.broadcast_to(e_mean, (NLOC, EH))],
                            axis=0).astype(np.float32)
        o = np.lexsort((sp, gr))  # by group, then src (gather locality)
        sp, gr, rr, ef = sp[o], gr[o], rr[o], ef[o]
        M = len(sp)
        gcnt = np.bincount(gr, minlength=NT)
        assert gcnt.max() <= CPG * 128
        goff = np.zeros(NT + 1, np.int64)
        goff[1:] = np.cumsum(gcnt)
        rank = np.arange(M) - goff[gr]
        pos = gr * (CPG * 128) + rank

        srcs = np.zeros(SLOTS, np.int16)
        srcs[pos] = sp.astype(np.int16)
        rv = np.full(SLOTS, -1.0, np.float32)
        rv[pos] = rr
        efs = np.zeros((SLOTS, EH), np.float32)
        efs[pos] = ef

        ecb = np.zeros((NCHE, 17, 128), np.float16)
        ecb[:NCH, :16, :] = efs.reshape(NCH, 128, EH).transpose(0, 2, 1)
        ecb[:NCH, 16, :] = 1.0
        ecb2 = (ecb.reshape(NCHE // 2, 2, 17, 128).transpose(0, 2, 1, 3)
                .reshape(NCHE // 2, 17, 256))

        # dma_gather index layout: idx j at [j%16, j//16], replicated x8
        idx16 = srcs.reshape(SLOTS // 16, 16).T
        idx128 = np.tile(idx16, (8, 1)).astype(np.int16)

        rrow = rv.reshape(1, SLOTS).astype(np.float16)
        rcolT = rv.reshape(NCH, 128).T.copy().astype(np.float32)

        gmask = np.zeros((NT, 128, B), np.float16)
        nodes_c = np.where(core_of_node == c)[0]
        gmask[grp_in_core[nodes_c], slot_of[nodes_c], batch[nodes_c]] = 1.0

        xT = np.zeros((KX * 128, NLOC), np.float16)
        own = grp_in_core[nodes_c] * 128 + slot_of[nodes_c]
        xT[:NODE_DIM, own] = x[nodes_c].T.astype(np.float16)
        xT[NODE_DIM, :] = 1.0  # bias row

        per_core.append(dict(xT=xT, ecb=ecb2, idx=idx128, rrow=rrow,
                             rcolT=rcolT, gmask=gmask))
        dbg.append(dict(srcs=srcs, rv=rv, efs=efs))

    # ---- shared weights
    f32 = np.float32
    W_in = np.asarray(inputs["W_in"], f32)
    b_in = np.asarray(inputs["b_in"], f32)
    winp = np.zeros((KX * 128, HID), f32)
    winp[:NODE_DIM] = W_in
    winp[NODE_DIM] = b_in
    winp = winp.reshape(KX, 128, HID).astype(np.float16)

    def rep(v):  # replicate a [HID] vector across partitions
        return np.broadcast_to(np.asarray(v, f32), (128, HID)).astype(np.float16).copy()

    Wl = np.asarray(inputs["Wl"], f32)
    Wr = np.asarray(inputs["Wr"], f32)
    bl = np.asarray(inputs["bl"], f32)
    br = np.asarray(inputs["br"], f32)
    We = np.asarray(inputs["We"], f32)
    att = np.asarray(inputs["att"], f32)
    bconv = np.asarray(inputs["bconv"], f32)
    ln_g = np.asarray(inputs["ln_g"], f32)
    ln_b = np.asarray(inputs["ln_b"], f32)

    wl = Wl.reshape(L, 2, 128, HID).astype(np.float16)
    wr = Wr.reshape(L, 2, 128, HID).astype(np.float16)
    weaug = np.zeros((L, 17, HID), f32)
    weaug[:, :16] = We
    weaug[:, 16] = bl + br
    weaug = weaug.astype(np.float16)
    attf = np.stack([rep(att[i].reshape(HID)) for i in range(L)])
    bconv2 = np.stack([rep(bconv[i] + bl[i]) for i in range(L)])
    lng = np.stack([rep(ln_g[i]) for i in range(L)])
    lnb = np.stack([rep(ln_b[i]) for i in range(L)])

    iota_row = np.broadcast_to(np.arange(128, dtype=np.float16),
                               (128, 128)).copy()
    iota_col = np.arange(128, dtype=np.float32).reshape(128, 1).copy()

    Wg1 = np.asarray(inputs["Wg1"], f32)  # (256,128)
    Wg2 = np.asarray(inputs["Wg2"], f32)  # (128,1)
    Wh1 = np.asarray(inputs["Wh1"], f32)  # (256,64)
    Wh2 = np.asarray(inputs["Wh2"], f32)  # (64,1)
    GW = Wg1.shape[1]
    HW1 = Wh1.shape[1]
    shared = dict(
        winp=winp,
        lnin_g=rep(inputs["ln_in_g"]), lnin_b=rep(inputs["ln_in_b"]),
        wl=wl, wr=wr, weaug=weaug, attf=attf, bconv2=bconv2, lng=lng, lnb=lnb,
        iota_row=iota_row, iota_col=iota_col,
        wg1=Wg1.reshape(2, 128, GW).astype(np.float16),
        bg1=np.asarray(inputs["bg1"], f32).reshape(1, GW).astype(np.float16),
        wg2=np.broadcast_to(Wg2.reshape(GW), (128, GW)).astype(np.float16).copy(),
        bg2=np.full((128, 1), float(np.asarray(inputs["bg2"]).reshape(())), f32),
        wh1=Wh1.reshape(2, 128, HW1).astype(np.float16),
        bh1=np.broadcast_to(np.asarray(inputs["bh1"], f32), (128, HW1)).astype(np.float16).copy(),
        wh2=np.broadcast_to(Wh2.reshape(HW1), (128, HW1)).astype(np.float16).copy(),
        bh2=np.full((128, 1), float(np.asarray(inputs["bh2"]).reshape(())), f32),
    )

    in_maps = []
    for c in range(NCORES):
        m = dict(shared)
        m.update(per_core[c])
        in_maps.append({k: np.ascontiguousarray(v) for k, v in m.items()})

    meta = dict(NLOC=NLOC, NT=NT, CPG=CPG, NCH=NCH, NCHE=NCHE, SLOTS=SLOTS,
                GLOB=GLOB, GW=GW, HW1=HW1, in_maps=in_maps, dbg=dbg)
    return meta


# ----------------------------------------------------------------------------
# device program
# ----------------------------------------------------------------------------
def build(meta, num_devices=NCORES, nlayers=L):
    NLOC, NT, CPG, NCH = meta["NLOC"], meta["NT"], meta["CPG"], meta["NCH"]
    NCHE, SLOTS, GW, HW1 = meta["NCHE"], meta["SLOTS"], meta["GW"], meta["HW1"]
    ICOLS = SLOTS // 16
    SUP = 8  # chunks per supergather (dma_gather fails above 1024 idxs/call)

    nc = bacc.Bacc("TRN2", target_bir_lowering=False, debug=False,
                   enable_asserts=True, num_devices=num_devices)

    def din(name, shape, dt=F16):
        return nc.dram_tensor(name, list(shape), dt, kind="ExternalInput").ap()

    # inputs
    xT_d = din("xT", (KX * 128, NLOC))
    winp_d = din("winp", (KX, 128, HID))
    lnin_g_d = din("lnin_g", (128, HID))
    lnin_b_d = din("lnin_b", (128, HID))
    wl_d = din("wl", (L, 2, 128, HID))
    wr_d = din("wr", (L, 2, 128, HID))
    weaug_d = din("weaug", (L, 17, HID))
    attf_d = din("attf", (L, 128, HID))
    bconv2_d = din("bconv2", (L, 128, HID))
    lng_d = din("lng", (L, 128, HID))
    lnb_d = din("lnb", (L, 128, HID))
    ecb_d = din("ecb", (NCHE // 2, 17, 256))
    idx_d = din("idx", (128, ICOLS), I16)
    rrow_d = din("rrow", (1, SLOTS))
    rcolT_d = din("rcolT", (128, NCH), F32)
    iota_row_d = din("iota_row", (128, 128))
    iota_col_d = din("iota_col", (128, 1), F32)
    gmask_d = din("gmask", (NT, 128, B))
    wg1_d = din("wg1", (2, 128, GW))
    bg1_d = din("bg1", (1, GW))
    wg2_d = din("wg2", (128, GW))
    bg2_d = din("bg2", (128, 1), F32)
    wh1_d = din("wh1", (2, 128, HW1))
    bh1_d = din("bh1", (128, HW1))
    wh2_d = din("wh2", (128, HW1))
    bh2_d = din("bh2", (128, 1), F32)
    y_d = nc.dram_tensor("y", [B, 1], F32, kind="ExternalOutput").ap()
    import os
    DEBUG = bool(os.environ.get("GNN_DEBUG"))
    VAR = os.environ.get("GNN_VAR", "")
    if DEBUG:
        dbg_sel_d = nc.dram_tensor("dbg_sel", [4, 128, 128], F16,
                                   kind="ExternalOutput").ap()
        dbg_xlg_d = nc.dram_tensor("dbg_xlg", [128, 8, HID], F16,
                                   kind="ExternalOutput").ap()
        dbg_h_d = nc.dram_tensor("dbg_h", [NLOC, HID], F16,
                                 kind="ExternalOutput").ap()
        dbg_pp_d = nc.dram_tensor("dbg_pp", [128, 48], F32,
                                  kind="ExternalOutput").ap()
        dbg_pq_d = nc.dram_tensor("dbg_pq", [128, 48], F32,
                                  kind="ExternalOutput").ap()
        dbg_eg_d = nc.dram_tensor("dbg_eg", [128, NT], F32,
                                  kind="ExternalOutput").ap()

    # internal DRAM
    hloc_d = nc.dram_tensor("hloc", [NLOC, HID], F16).ap()
    xlb_d = nc.dram_tensor("xlb", [NLOC, HID], F16).ap()
    xla_d = [nc.dram_tensor(f"xla{i}", [NCORES * NLOC, HID], F16,
                            addr_space="Shared").ap() for i in range(nlayers)]
    poolb_d = nc.dram_tensor("poolb", [128, 48], F32).ap()
    pools_d = nc.dram_tensor("pools", [128, 48], F32,
                             addr_space="Shared").ap()

    rg = [list(range(num_devices))]

    with tile.TileContext(nc) as tc:
        import contextlib
        ctx = contextlib.ExitStack()
        with ctx:
            const = ctx.enter_context(tc.tile_pool(name="const", bufs=1))
            work = ctx.enter_context(tc.tile_pool(name="work", bufs=3))
            small = ctx.enter_context(tc.tile_pool(name="small", bufs=4))
            xtp = ctx.enter_context(tc.tile_pool(name="xtp", bufs=3))
            scp = ctx.enter_context(tc.tile_pool(name="scp", bufs=3))
            xlg_p = ctx.enter_context(tc.tile_pool(name="xlg", bufs=2))
            ps_mm = ctx.enter_context(tc.tile_pool(name="ps_mm", bufs=2, space="PSUM"))
            ps_ed = ctx.enter_context(tc.tile_pool(name="ps_ed", bufs=2, space="PSUM"))
            ps_ag = ctx.enter_context(tc.tile_pool(name="ps_ag", bufs=2, space="PSUM"))

            # ---------------- resident tiles
            winp_t = const.tile([128, KX, HID], F16)
            nc.sync.dma_start(out=winp_t[:], in_=winp_d.rearrange("k p f -> p k f"))
            lnin_g_t = const.tile([128, HID], F16)
            nc.sync.dma_start(out=lnin_g_t[:], in_=lnin_g_d[:])
            lnin_b_t = const.tile([128, HID], F16)
            nc.sync.dma_start(out=lnin_b_t[:], in_=lnin_b_d[:])
            wl_t = const.tile([128, L, 2, HID], F16)
            nc.sync.dma_start(out=wl_t[:], in_=wl_d.rearrange("l k p f -> p l k f"))
            wr_t = const.tile([128, L, 2, HID], F16)
            nc.sync.dma_start(out=wr_t[:], in_=wr_d.rearrange("l k p f -> p l k f"))
            weaug_t = const.tile([17, L, HID], F16)
            nc.sync.dma_start(out=weaug_t[:], in_=weaug_d.rearrange("l p f -> p l f"))
            attf_t = const.tile([128, L, HID], F16)
            nc.sync.dma_start(out=attf_t[:], in_=attf_d.rearrange("l p f -> p l f"))
            bconv2_t = const.tile([128, L, HID], F16)
            nc.sync.dma_start(out=bconv2_t[:], in_=bconv2_d.rearrange("l p f -> p l f"))
            lng_t = const.tile([128, L, HID], F16)
            nc.sync.dma_start(out=lng_t[:], in_=lng_d.rearrange("l p f -> p l f"))
            lnb_t = const.tile([128, L, HID], F16)
            nc.sync.dma_start(out=lnb_t[:], in_=lnb_d.rearrange("l p f -> p l f"))
            idx_t = const.tile([128, ICOLS], I16)
            nc.sync.dma_start(out=idx_t[:], in_=idx_d[:])
            rrow_t = const.tile([1, SLOTS], F16)
            nc.sync.dma_start(out=rrow_t[:], in_=rrow_d[:])
            rcolT_t = const.tile([128, NCH], F32)
            nc.sync.dma_start(out=rcolT_t[:], in_=rcolT_d[:])
            iota_row_t = const.tile([128, 128], F16)
            nc.sync.dma_start(out=iota_row_t[:], in_=iota_row_d[:])
            iota_col_t = const.tile([128, 1], F32)
            nc.sync.dma_start(out=iota_col_t[:], in_=iota_col_d[:])
            gmask_t = const.tile([128, NT, B], F16)
            nc.sync.dma_start(out=gmask_t[:], in_=gmask_d.rearrange("t p g -> p t g"))
            wg1_t = const.tile([128, 2, GW], F16)
            nc.sync.dma_start(out=wg1_t[:], in_=wg1_d.rearrange("k p f -> p k f"))
            bg1_t = const.tile([1, GW], F16)
            nc.sync.dma_start(out=bg1_t[:], in_=bg1_d[:])
            wg2_t = const.tile([128, GW], F16)
            nc.sync.dma_start(out=wg2_t[:], in_=wg2_d[:])
            bg2_t = const.tile([128, 1], F32)
            nc.sync.dma_start(out=bg2_t[:], in_=bg2_d[:])
            wh1_t = const.tile([128, 2, HW1], F16)
            nc.sync.dma_start(out=wh1_t[:], in_=wh1_d.rearrange("k p f -> p k f"))
            bh1_t = const.tile([128, HW1], F16)
            nc.sync.dma_start(out=bh1_t[:], in_=bh1_d[:])
            wh2_t = const.tile([128, HW1], F16)
            nc.sync.dma_start(out=wh2_t[:], in_=wh2_d[:])
            bh2_t = const.tile([128, 1], F32)
            nc.sync.dma_start(out=bh2_t[:], in_=bh2_d[:])

            h_res = const.tile([128, NT, HID + 1], F16)
            hT_loc = const.tile([128, 2, NLOC], F16)
            xr_t = const.tile([128, NT, HID], F16)
            ones1_t = const.tile([1, 128], F16)
            nc.vector.memset(ones1_t[:], 1.0)
            eps_t = const.tile([128, 1], F32)
            nc.vector.memset(eps_t[:], 1e-5)
            expb_t = const.tile([128, 1], F32)
            nc.vector.memset(expb_t[:], EXP_BIAS)
            gateb_t = const.tile([128, 1], F32)
            nc.vector.memset(gateb_t[:], GATE_BIAS)
            for t in range(NT):
                nc.vector.memset(h_res[:, t, HID:HID + 1], 1.0)

            def refine_recip(r_ap, x_ap, shape, tag):
                # r <- r*(2 - x*r), one Newton step on a LUT seed
                t = small.tile(shape, F32, tag=tag)
                nc.vector.tensor_tensor(out=t[:], in0=x_ap, in1=r_ap, op=OP.mult)
                nc.vector.tensor_scalar(out=t[:], in0=t[:], scalar1=2.0,
                                        scalar2=-1.0, op0=OP.subtract, op1=OP.mult)
                nc.vector.tensor_tensor(out=r_ap, in0=r_ap, in1=t[:], op=OP.mult)

            def refine_rsqrt(r_ap, x_ap, shape, tag):
                # r <- 0.5*r*(3 - x*r*r)
                t = small.tile(shape, F32, tag=tag)
                nc.vector.tensor_tensor(out=t[:], in0=r_ap, in1=r_ap, op=OP.mult)
                nc.vector.tensor_tensor(out=t[:], in0=x_ap, in1=t[:], op=OP.mult)
                nc.vector.tensor_scalar(out=t[:], in0=t[:], scalar1=3.0,
                                        scalar2=-0.5, op0=OP.subtract, op1=OP.mult)
                nc.vector.tensor_tensor(out=r_ap, in0=r_ap, in1=t[:], op=OP.mult)

            # ---------------- LN helper: s_t fp16 [128,HID] + musum f32 -> dest
            def layernorm(s_t, musum, g_ap, b_ap, dest_ap, gelu_after=False):
                mu = small.tile([128, 1], F32, tag="mu")
                nc.vector.tensor_scalar(out=mu[:], in0=musum, scalar1=1.0 / HID,
                                        scalar2=None, op0=OP.mult)
                d_t = work.tile([128, HID], F16, tag="d")
                nc.vector.tensor_scalar(out=d_t[:], in0=s_t, scalar1=mu[:],
                                        scalar2=None, op0=OP.subtract)
                scr = work.tile([128, HID], F16, tag="scr")
                vs = small.tile([128, 1], F32, tag="vs")
                nc.vector.tensor_tensor(out=scr[:], in0=d_t[:], in1=d_t[:], op=OP.mult)
                nc.vector.tensor_reduce(out=vs[:], in_=scr[:],
                                        axis=mybir.AxisListType.X, op=OP.add)
                vx = small.tile([128, 1], F32, tag="vx")
                nc.vector.tensor_scalar(out=vx[:], in0=vs[:], scalar1=1.0 / HID,
                                        scalar2=None, op0=OP.mult)
                nc.vector.tensor_scalar(out=vx[:], in0=vx[:], scalar1=eps_t[:],
                                        scalar2=None, op0=OP.add)
                sd = small.tile([128, 1], F32, tag="sd")
                nc.scalar.activation(out=sd[:], in_=vx[:], func=AF.Ln)
                rstd = small.tile([128, 1], F32, tag="rstd")
                nc.scalar.activation(out=rstd[:], in_=sd[:], func=AF.Exp, scale=-0.5)
                refine_rsqrt(rstd[:], vx[:], [128, 1], "nsr")
                n_t = work.tile([128, HID], F16, tag="n")
                nc.vector.tensor_scalar(out=n_t[:], in0=d_t[:], scalar1=rstd[:],
                                        scalar2=None, op0=OP.mult)
                nc.vector.tensor_tensor(out=n_t[:], in0=n_t[:], in1=g_ap, op=OP.mult)
                if gelu_after:
                    nc.vector.tensor_tensor(out=n_t[:], in0=n_t[:], in1=b_ap, op=OP.add)
                    nc.scalar.activation(out=dest_ap, in_=n_t[:], func=AF.Gelu)
                else:
                    nc.vector.tensor_tensor(out=dest_ap, in0=n_t[:], in1=b_ap, op=OP.add)

            # ---------------- phase A: input projection (local nodes)
            for t2 in range((NT + 1) // 2):
                tcnt = min(2, NT - t2 * 2)
                xt_t = xtp.tile([128, KX, 2 * 128], F16, tag="xt")
                for k in range(KX):
                    nc.sync.dma_start(
                        out=xt_t[:, k, :tcnt * 128],
                        in_=xT_d[k * 128:(k + 1) * 128,
                                 t2 * 256:t2 * 256 + tcnt * 128])
                for j in range(tcnt):
                    t = t2 * 2 + j
                    ps = ps_mm.tile([128, HID], F32, tag="mmps")
                    for k in range(KX):
                        nc.tensor.matmul(out=ps[:], lhsT=xt_t[:, k, j * 128:(j + 1) * 128],
                                         rhs=winp_t[:, k, :], start=(k == 0),
                                         stop=(k == KX - 1))
                    s_t = work.tile([128, HID], F16, tag="s")
                    musum = small.tile([128, 1], F32, tag="musum")
                    nc.scalar.activation(out=s_t[:], in_=ps[:], func=AF.Copy,
                                         accum_out=musum[:])
                    layernorm(s_t[:], musum[:], lnin_g_t[:], lnin_b_t[:],
                              h_res[:, t, :HID], gelu_after=True)
                    nc.sync.dma_start(out=hloc_d[t * 128:(t + 1) * 128, :],
                                      in_=h_res[:, t, :HID])

            # ---------------- per layer
            def compute_hT():
                # hloc (node-major, HBM) -> hT_loc (feature-major, SBUF)
                for half in range(2):
                    nc.sync.dma_start(out=hT_loc[:, half, :],
                                      in_=hloc_d[:, half * 128:(half + 1) * 128],
                                      transpose=True)

            compute_hT()

            for i in range(nlayers):
                # local xl -> bounce -> AllGather into shared node-major table
                for t in range(NT):
                    ps = ps_mm.tile([128, HID], F32, tag="mmps")
                    for half in range(2):
                        nc.tensor.matmul(out=ps[:],
                                         lhsT=hT_loc[:, half, t * 128:(t + 1) * 128],
                                         rhs=wl_t[:, i, half, :],
                                         start=(half == 0), stop=(half == 1))
                    xl_t = work.tile([128, HID], F16, tag="xlt")
                    nc.scalar.activation(out=xl_t[:], in_=ps[:], func=AF.Copy)
                    nc.sync.dma_start(out=xlb_d[t * 128:(t + 1) * 128, :],
                                      in_=xl_t[:])
                if VAR != "nocoll":
                    nc.gpsimd.collective_compute(
                        "AllGather", OP.bypass, replica_groups=rg,
                        ins=[xlb_d[:]], outs=[xla_d[i][:]])
                # local xr (overlaps the collective)
                for t in range(NT):
                    ps = ps_mm.tile([128, HID], F32, tag="mmps")
                    for half in range(2):
                        nc.tensor.matmul(out=ps[:],
                                         lhsT=hT_loc[:, half, t * 128:(t + 1) * 128],
                                         rhs=wr_t[:, i, half, :],
                                         start=(half == 0), stop=(half == 1))
                    nc.scalar.activation(out=xr_t[:, t, :], in_=ps[:], func=AF.Copy)

                # edge phase
                xlg_tiles = {}
                agg = None
                for chk in range(NCH):
                    s, joff = divmod(chk, SUP)
                    if VAR == "nogather" and s > 0:
                        s = 0
                    if joff == 0 and (VAR != "nogather" or s == 0 or True) and not (VAR == "nogather" and s != chk // SUP):
                        cnt = min(SUP, NCH - s * SUP)
                        xlg = xlg_p.tile([128, SUP, HID], F16, tag="xlg")
                        nc.gpsimd.dma_gather(
                            out_ap=xlg[:, :cnt, :], in_ap=xla_d[i][:, :],
                            idxs_ap=idx_t[:, s * (SUP * 8):s * (SUP * 8) + cnt * 8],
                            num_idxs=cnt * 128, num_idxs_reg=cnt * 128,
                            elem_size=HID)
                        xlg_tiles[s] = xlg
                        if DEBUG and i == 0 and s == 0:
                            nc.sync.dma_start(out=dbg_xlg_d[:], in_=xlg[:, :8, :])
                    xlg = xlg_tiles[s]
                    g, cidx = divmod(chk, CPG)

                    if chk % 2 == 0 and (VAR != "noec" or chk == 0):
                        ec2 = scp.tile([17, 256], F16, tag="ec2")
                        nc.sync.dma_start(out=ec2[:], in_=ecb_d[chk // 2])
                        ec2_cur = ec2
                    half = (chk % 2) * 128

                    # on-device selector one-hots (Pool engine)
                    if VAR != "nosel" or chk == 0:
                        bc_t = scp.tile([128, 128], F16, tag="bc")
                        nc.gpsimd.partition_broadcast(
                            bc_t[:], rrow_t[0:1, chk * 128:(chk + 1) * 128])
                        sct_t = scp.tile([128, 128], F16, tag="sct")
                        nc.gpsimd.tensor_scalar(out=sct_t[:], in0=iota_row_t[:],
                                                scalar1=rcolT_t[:, chk:chk + 1],
                                                scalar2=None, op0=OP.is_equal)
                        scb_t = scp.tile([128, 128], F16, tag="scb")
                        nc.gpsimd.tensor_scalar(out=scb_t[:], in0=bc_t[:],
                                                scalar1=iota_col_t[:],
                                                scalar2=None, op0=OP.is_equal)
                        sel_keep = (sct_t, scb_t)
                    else:
                        sct_t, scb_t = sel_keep
                    if DEBUG and i == 0 and chk < 2:
                        nc.sync.dma_start(out=dbg_sel_d[2 * chk], in_=scb_t[:])
                        nc.sync.dma_start(out=dbg_sel_d[2 * chk + 1], in_=sct_t[:])

                    ps = ps_ed.tile([128, HID], F32, tag="edps")
                    nc.tensor.matmul(out=ps[:], lhsT=scb_t[:],
                                     rhs=xr_t[:, g, :], start=True, stop=False)
                    nc.tensor.matmul(out=ps[:], lhsT=ec2_cur[:, half:half + 128],
                                     rhs=weaug_t[:, i, :], start=False, stop=True)
                    m_t = work.tile([128, HID], F16, tag="m")
                    nc.vector.tensor_tensor(out=m_t[:], in0=xlg[:, joff, :],
                                            in1=ps[:], op=OP.add)
                    lr_t = work.tile([128, HID], F16, tag="lr")
                    nc.scalar.activation(out=lr_t[:], in_=m_t[:], func=AF.Copy,
                                         scale=0.2)
                    nc.vector.tensor_tensor(out=m_t[:], in0=m_t[:], in1=lr_t[:],
                                            op=OP.max)
                    v_t = work.tile([128, HID], F16, tag="v")
                    nc.vector.tensor_tensor(out=v_t[:], in0=m_t[:],
                                            in1=attf_t[:, i, :], op=OP.mult)
                    a_t = small.tile([128, H], F32, tag="a")
                    nc.vector.tensor_reduce(
                        out=a_t[:], in_=v_t[:].rearrange("p (h d) -> p h d", d=DH),
                        axis=mybir.AxisListType.X, op=OP.add)
                    u_t = work.tile([128, HID + H], F16, tag="u")
                    nc.scalar.activation(out=u_t[:, HID:HID + H], in_=a_t[:],
                                         func=AF.Exp, bias=expb_t[:])
                    nc.vector.tensor_tensor(
                        out=u_t[:, :HID].rearrange("p (h d) -> p h d", d=DH),
                        in0=xlg[:, joff, :].rearrange("p (h d) -> p h d", d=DH),
                        in1=u_t[:, HID:HID + H].to_broadcast([128, H, DH]),
                        op=OP.mult)
                    if cidx == 0:
                        agg = ps_ag.tile([128, HID + H], F32, tag="agg")
                    nc.tensor.matmul(out=agg[:], lhsT=sct_t[:],
                                     rhs=u_t[:], start=(cidx == 0),
                                     stop=(cidx == CPG - 1))

                    if cidx == CPG - 1:
                        rd = small.tile([128, H], F32, tag="rd")
                        nc.scalar.activation(out=rd[:], in_=agg[:, HID:HID + H],
                                             func=AF.Ln)
                        nc.scalar.activation(out=rd[:], in_=rd[:], func=AF.Exp,
                                             scale=-1.0)
                        refine_recip(rd[:], agg[:, HID:HID + H], [128, H], "nrd")
                        o_t = work.tile([128, HID], F16, tag="o")
                        nc.vector.tensor_tensor(
                            out=o_t[:].rearrange("p (h d) -> p h d", d=DH),
                            in0=agg[:, :HID].rearrange("p (h d) -> p h d", d=DH),
                            in1=rd[:].to_broadcast([128, H, DH]), op=OP.mult)
                        nc.vector.tensor_tensor(out=o_t[:], in0=o_t[:],
                                                in1=bconv2_t[:, i, :], op=OP.add)
                        nc.scalar.activation(out=o_t[:], in_=o_t[:], func=AF.Gelu)
                        s_t = work.tile([128, HID], F16, tag="s")
                        musum = small.tile([128, 1], F32, tag="musum")
                        nc.vector.tensor_tensor(out=s_t[:], in0=o_t[:],
                                                in1=h_res[:, g, :HID], op=OP.add)
                        nc.vector.tensor_reduce(out=musum[:], in_=s_t[:],
                                                axis=mybir.AxisListType.X, op=OP.add)
                        layernorm(s_t[:], musum[:], lng_t[:, i, :], lnb_t[:, i, :],
                                  h_res[:, g, :HID])
                        nc.sync.dma_start(out=hloc_d[g * 128:(g + 1) * 128, :],
                                          in_=h_res[:, g, :HID])
                compute_hT()

            if DEBUG:
                for t in range(NT):
                    hcp = work.tile([128, HID], F16, tag="hcp")
                    nc.sync.dma_start(out=hcp[:],
                                      in_=hloc_d[t * 128:(t + 1) * 128, :])
                    nc.sync.dma_start(out=dbg_h_d[t * 128:(t + 1) * 128, :],
                                      in_=hcp[:])

            # ---------------- pooling (transposed accumulation) + head
            wm_all = const.tile([128, NT, B], F16)
            for t in range(NT):
                g1 = ps_mm.tile([128, HID], F32, tag="mmps")
                for half in range(2):
                    nc.tensor.matmul(out=g1[:, :GW],
                                     lhsT=hT_loc[:, half, t * 128:(t + 1) * 128],
                                     rhs=wg1_t[:, half, :], start=(half == 0),
                                     stop=False)
                nc.tensor.matmul(out=g1[:, :GW], lhsT=ones1_t[:],
                                 rhs=bg1_t[:], start=False, stop=True)
                t_t = work.tile([128, GW], F16, tag="tt")
                nc.scalar.activation(out=t_t[:], in_=g1[:, :GW], func=AF.Tanh)
                scr = work.tile([128, GW], F16, tag="scr2")
                gate = small.tile([128, 1], F32, tag="gate")
                nc.vector.tensor_tensor(out=scr[:], in0=t_t[:], in1=wg2_t[:],
                                        op=OP.mult)
                nc.vector.tensor_reduce(out=gate[:], in_=scr[:],
                                        axis=mybir.AxisListType.X, op=OP.add)
                nc.vector.tensor_scalar(out=gate[:], in0=gate[:], scalar1=bg2_t[:],
                                        scalar2=None, op0=OP.add)
                eg = small.tile([128, 1], F16, tag="eg")
                nc.scalar.activation(out=eg[:], in_=gate[:], func=AF.Exp,
                                     bias=gateb_t[:])
                if DEBUG:
                    egd = small.tile([128, 1], F32, tag="egd")
                    nc.vector.tensor_scalar(out=egd[:], in0=eg[:], scalar1=1.0,
                                            scalar2=None, op0=OP.mult)
                    nc.sync.dma_start(out=dbg_eg_d[:, t:t + 1], in_=egd[:])
                nc.vector.tensor_tensor(out=wm_all[:, t, :], in0=gmask_t[:, t, :],
                                        in1=eg[:].to_broadcast([128, B]), op=OP.mult)
            # three sequential single-group accumulation passes: a matmul
            # start=True resets its whole PSUM bank, so groups must not
            # interleave within a bank.
            pp = work.tile([128, 48], F32, tag="pp")
            nc.vector.memset(pp[:], 0.0)
            lhss = [(slice(0, 128), (0, 16), 128),
                    (slice(128, 256), (16, 32), 128),
                    (slice(HID, HID + 1), (32, 48), 1)]
            for lh, (c0, c1), mrows in lhss:
                psp = ps_ag.tile([128, HID + H], F32, tag="agg")
                for t in range(NT):
                    nc.tensor.matmul(out=psp[:mrows, 0:16],
                                     lhsT=h_res[:, t, lh],
                                     rhs=wm_all[:, t, :],
                                     start=(t == 0), stop=(t == NT - 1))
                nc.scalar.activation(out=pp[:mrows, c0:c1], in_=psp[:mrows, 0:16],
                                     func=AF.Copy)
            nc.sync.dma_start(out=poolb_d[:], in_=pp[:])
            if DEBUG:
                nc.sync.dma_start(out=dbg_pp_d[:], in_=pp[:])
            nc.gpsimd.collective_compute(
                "AllReduce", OP.add, replica_groups=rg,
                ins=[poolb_d[:]], outs=[pools_d[:]])
            pq = work.tile([128, 48], F32, tag="pq")
            nc.sync.dma_start(out=pq[:], in_=pools_d[:])
            if DEBUG:
                nc.sync.dma_start(out=dbg_pq_d[:], in_=pq[:])
            # reciprocal of the per-graph denominators (row 0, cols 32:48)
            rdp = small.tile([1, 16], F32, tag="rdp")
            nc.scalar.activation(out=rdp[:], in_=pq[0:1, 32:48], func=AF.Ln)
            nc.scalar.activation(out=rdp[:], in_=rdp[:], func=AF.Exp, scale=-1.0)
            refine_recip(rdp[:], pq[0:1, 32:48], [1, 16], "nrp")
            rdb = small.tile([128, 16], F32, tag="rdb")
            nc.gpsimd.partition_broadcast(rdb[:], rdp[:])
            pooledT = work.tile([128, 2, 16], F16, tag="pooledT")
            for half in range(2):
                nc.vector.tensor_tensor(out=pooledT[:, half, :],
                                        in0=pq[:, half * 16:(half + 1) * 16],
                                        in1=rdb[:], op=OP.mult)
            o1ps = ps_mm.tile([128, HID], F32, tag="mmps")
            for half in range(2):
                nc.tensor.matmul(out=o1ps[0:16, 0:HW1], lhsT=pooledT[:, half, :],
                                 rhs=wh1_t[:, half, :], start=(half == 0),
                                 stop=(half == 1))
            o1 = work.tile([16, HW1], F16, tag="o1s")
            nc.vector.tensor_tensor(out=o1[:], in0=o1ps[0:16, 0:HW1],
                                    in1=bh1_t[0:16, :], op=OP.add)
            nc.scalar.activation(out=o1[:], in_=o1[:], func=AF.Gelu)
            scr3 = work.tile([16, HW1], F16, tag="scr3")
            yv = small.tile([16, 1], F32, tag="yv")
            nc.vector.tensor_tensor(out=scr3[:], in0=o1[:], in1=wh2_t[0:16, :],
                                    op=OP.mult)
            nc.vector.tensor_reduce(out=yv[:], in_=scr3[:],
                                    axis=mybir.AxisListType.X, op=OP.add)
            nc.vector.tensor_scalar(out=yv[:], in0=yv[:], scalar1=bh2_t[0:16, :],
                                    scalar2=None, op0=OP.add)
            nc.sync.dma_start(out=y_d[:], in_=yv[:])

    nc.compile()
    return nc


# ----------------------------------------------------------------------------
# persistent execution layer: compile once, keep inputs device-resident, so
# repeat executions measure kernel time rather than PJRT re-trace + re-stage.
# Mirrors concourse.bass2jax.run_bass_via_pjrt's lowering contract exactly.
# ----------------------------------------------------------------------------
def _make_runner(nc, in_maps, n_cores):
    import jax
    from jax.experimental.shard_map import shard_map
    from jax.sharding import Mesh, PartitionSpec, NamedSharding
    from concourse import bass2jax

    bass2jax.install_neuronx_cc_hook()

    if nc.dbg_addr is not None:
        if nc.dbg_callbacks:
            raise RuntimeError("dbg callbacks unsupported in persistent runner")
        in_maps = [{**m, nc.dbg_addr.name: np.zeros((1, 2), np.uint32)}
                   for m in in_maps]

    partition_name = nc.partition_id_tensor.name if nc.partition_id_tensor else None
    in_names, out_names, out_avals, zero_outs = [], [], [], []
    for alloc in nc.m.functions[0].allocations:
        if not isinstance(alloc, mybir.MemoryLocationSet):
            continue
        name = alloc.memorylocations[0].name
        if alloc.kind == "ExternalInput":
            if name != partition_name:
                in_names.append(name)
        elif alloc.kind == "ExternalOutput":
            shape = tuple(alloc.tensor_shape)
            dtype = mybir.dt.np(alloc.dtype)
            out_names.append(name)
            out_avals.append(jax.core.ShapedArray(shape, dtype))
            zero_outs.append(np.zeros(shape, dtype))
    n_params = len(in_names)
    n_outs = len(out_avals)
    in_names_all = list(in_names) + out_names
    if partition_name is not None:
        in_names_all.append(partition_name)

    def _body(*args):
        operands = list(args)
        if partition_name is not None:
            operands.append(bass2jax.partition_id_tensor())
        outs = bass2jax._bass_exec_p.bind(
            *operands,
            out_avals=tuple(out_avals),
            in_names=tuple(in_names_all),
            out_names=tuple(out_names),
            lowering_input_output_aliases=(),
            sim_require_finite=True,
            sim_require_nnan=True,
            nc=nc,
        )
        return tuple(outs)

    devices = jax.devices()[:n_cores]
    mesh = Mesh(np.asarray(devices), ("core",))
    in_specs = (PartitionSpec("core"),) * (n_params + n_outs)
    out_specs = (PartitionSpec("core"),) * n_outs
    fn = shard_map(_body, mesh=mesh, in_specs=in_specs, out_specs=out_specs,
                   check_rep=False)

    per_core = [[np.asarray(m[name]) for name in in_names] for m in in_maps]
    concat_in = [np.concatenate([per_core[c][i] for c in range(n_cores)], axis=0)
                 for i in range(n_params)]
    sh = NamedSharding(mesh, PartitionSpec("core"))
    dev_in = [jax.device_put(a, sh) for a in concat_in]

    # No donation: the kernel fully writes its ExternalOutputs, so the
    # zero "output seed" buffers can stay resident across calls.
    zshapes = [(n_cores * z.shape[0], *z.shape[1:]) for z in zero_outs]
    zdtypes = [z.dtype for z in zero_outs]
    dev_zeros = [jax.device_put(np.zeros(s, d), sh)
                 for s, d in zip(zshapes, zdtypes)]
    compiled = bass2jax.fast_dispatch_compile(
        lambda: jax.jit(fn, keep_unused=True)
        .lower(*dev_in, *dev_zeros).compile())
    global _LAST_RUNNER
    _LAST_RUNNER = (compiled, dev_in, dev_zeros)

    def run():
        outs = jax.block_until_ready(compiled(*dev_in, *dev_zeros))
        return [
            {name: np.asarray(outs[i]).reshape(n_cores, *out_avals[i].shape)[c]
             for i, name in enumerate(out_names)}
            for c in range(n_cores)
        ]
    return run


# ----------------------------------------------------------------------------
# entry point
# ----------------------------------------------------------------------------
LAST_EXEC_NS = None
_LAST = {}
_LAST_RUNNER = None


def rerun(n=3):
    """Re-execute the already-built program; returns min wall seconds."""
    import time
    run = _LAST["run"]
    best = float("inf")
    for _ in range(n):
        t0 = time.time()
        run()
        best = min(best, time.time() - t0)
    return best


def exec_time_s(k=20, warm=3):
    """Steady-state per-execution time: marginal wall of pipelined launches."""
    import time
    import jax
    compiled, dev_in, dev_zeros = _LAST_RUNNER
    for _ in range(warm):
        jax.block_until_ready(compiled(*dev_in, *dev_zeros))
    t0 = time.time()
    outs = None
    for _ in range(k):
        outs = compiled(*dev_in, *dev_zeros)
    jax.block_until_ready(outs)
    t_k = time.time() - t0
    t0 = time.time()
    outs = None
    for _ in range(2 * k):
        outs = compiled(*dev_in, *dev_zeros)
    jax.block_until_ready(outs)
    t_2k = time.time() - t0
    return max((t_2k - t_k) / k, 1e-9)


def kernel(**inputs):
    global LAST_EXEC_NS
    from concourse.bass_interp import get_hw_module

    meta = prepare(inputs)
    nc = build(meta)
    nc.m = get_hw_module(nc.m)
    run = _make_runner(nc, meta["in_maps"], NCORES)
    results = run()
    _LAST.update(nc=nc, meta=meta, run=run)
    return results[0]["y"].reshape(B).astype(np.float32).copy()


# revision 23
# speedup vs baseline: 1815.2977x; 2.1788x over previous
"""Trainium2 Bass kernel for nn_BindingGNN (GATv2-style message-passing GNN).

Strategy (8 NeuronCores, SPMD, single NEFF):
  - Nodes assigned freely to 8 cores x NT groups of 128 slots by LPT
    bin-packing on (indegree+1), so every group owns ~equal edge work and
    CPG (chunks per group) is minimal and uniform.
  - Edges owned by the core of their dst node, grouped under the dst's
    128-slot group, sorted by src within each group (gather locality),
    packed into 128-edge chunks (CPG chunks per group).
  - Per layer: xl computed locally (NT tiles), AllGathered node-major into
    a shared [NCORES*NLOC, HID] table; per-edge gather of xl rows via
    dma_gather; xr-expand / segment-sum selectors are 0/1 one-hots built
    ON DEVICE per chunk from tiny fp16 index rows (partition_broadcast +
    is_equal on the Pool engine) instead of 64KB/chunk HBM blobs;
    edge-feature projection via small matmul from a packed [17,256] blob;
    attention softmax without max-subtraction (constant bias for range
    safety); segment-sum aggregation via selector matmuls into PSUM.
  - Pooling: per-core partial attention-pool accumulated transposed
    ([feat, graph]) via matmuls, AllReduced ([128,48] f32), head computed
    redundantly on every core; host reads core 0.
Everything is fp16 on-chip with f32 PSUM/statistics.
"""
import sys
import numpy as np

sys.path.insert(0, "/opt/trn_rl_repo")

import concourse.bass as bass  # noqa: E402
import concourse.bacc as bacc  # noqa: E402
import concourse.tile as tile  # noqa: E402
from concourse import mybir  # noqa: E402
from concourse.masks import make_identity  # noqa: E402

F16 = mybir.dt.float16
F32 = mybir.dt.float32
I16 = mybir.dt.int16
AF = mybir.ActivationFunctionType
OP = mybir.AluOpType

HID = 256
NODE_DIM = 1280
L = 4
H = 4
DH = 64
EH = 16
B = 16
NCORES = 8
KX = 11  # ceil((1280+1)/128)
EXP_BIAS = -3.0
GATE_BIAS = -2.0


# ----------------------------------------------------------------------------
# host-side math (edge MLP is static per-edge preprocessing)
# ----------------------------------------------------------------------------
def _erf(x):
    try:
        from scipy.special import erf
        return erf(x)
    except Exception:
        import math
        v = np.vectorize(math.erf)
        return v(x).astype(x.dtype)


def _gelu_np(x):
    x64 = x.astype(np.float64)
    return (0.5 * x64 * (1.0 + _erf(x64 / np.sqrt(2.0)))).astype(np.float32)


def _edge_mlp_host(edge_attr, W_e1, b_e1, W_e2, b_e2):
    e = _gelu_np(edge_attr @ W_e1 + b_e1) @ W_e2 + b_e2
    return e.astype(np.float32)


# ----------------------------------------------------------------------------
# host-side sharding / blob construction
# ----------------------------------------------------------------------------
def prepare(inputs):
    import heapq
    x = np.asarray(inputs["x"], np.float32)
    edge_index = np.asarray(inputs["edge_index"]).astype(np.int64)
    batch = np.asarray(inputs["batch"]).astype(np.int64)
    N = x.shape[0]

    e_feat = _edge_mlp_host(np.asarray(inputs["edge_attr"], np.float32),
                            np.asarray(inputs["W_e1"], np.float32),
                            np.asarray(inputs["b_e1"], np.float32),
                            np.asarray(inputs["W_e2"], np.float32),
                            np.asarray(inputs["b_e2"], np.float32))
    e_mean = e_feat.mean(0)

    # ---- balanced node -> (core, group, slot) assignment (LPT on indeg+1)
    NT = -(-N // (NCORES * 128))
    NLOC = NT * 128
    NGRP = NCORES * NT
    GLOB = NCORES * NLOC
    assert GLOB < 32768, "padded node table must fit int16 indices"

    indeg = np.bincount(edge_index[1], minlength=N)
    w = indeg + 1
    heap = [(0, g) for g in range(NGRP)]
    heapq.heapify(heap)
    counts = np.zeros(NGRP, np.int64)
    group_of = np.empty(N, np.int64)
    slot_of = np.empty(N, np.int64)
    for n in np.argsort(-w, kind="stable"):
        while True:
            load, g = heapq.heappop(heap)
            if counts[g] < 128:
                break
        group_of[n] = g
        slot_of[n] = counts[g]
        counts[g] += 1
        heapq.heappush(heap, (load + int(w[n]), g))
    loads = np.zeros(NGRP, np.int64)
    np.add.at(loads, group_of, w)
    gtot = loads + (128 - counts)  # pad slots contribute one self-loop each
    CPG = int(-(-gtot.max() // 128))
    NCH = NT * CPG
    SLOTS = NCH * 128

    core_of_node = group_of // NT
    grp_in_core = group_of % NT
    padded_id = core_of_node * NLOC + grp_in_core * 128 + slot_of

    src_all = padded_id[edge_index[0]]
    dst_core = core_of_node[edge_index[1]]
    dst_grp = grp_in_core[edge_index[1]]
    dst_r = slot_of[edge_index[1]]

    NCHE = NCH + (NCH % 2)
    per_core = []
    dbg = []
    for c in range(NCORES):
        sel = dst_core == c
        sp = src_all[sel]
        gr = dst_grp[sel]
        rr = dst_r[sel]
        ef = e_feat[sel]
        # self-loops for every slot (including padded slots: keeps den>0)
        sp = np.concatenate([sp, c * NLOC + np.arange(NLOC)])
        gr = np.concatenate([gr, np.arange(NLOC) // 128])
        rr = np.concatenate([rr, np.arange(NLOC) % 128])
        ef = np.concatenate([ef, np.broadcast_to(e_mean, (NLOC, EH))],
                            axis=0).astype(np.float32)
        o = np.lexsort((sp, gr))  # by group, then src (gather locality)
        sp, gr, rr, ef = sp[o], gr[o], rr[o], ef[o]
        M = len(sp)
        gcnt = np.bincount(gr, minlength=NT)
        assert gcnt.max() <= CPG * 128
        goff = np.zeros(NT + 1, np.int64)
        goff[1:] = np.cumsum(gcnt)
        rank = np.arange(M) - goff[gr]
        pos = gr * (CPG * 128) + rank

        srcs = np.zeros(SLOTS, np.int16)
        srcs[pos] = sp.astype(np.int16)
        rv = np.full(SLOTS, -1.0, np.float32)
        rv[pos] = rr
        efs = np.zeros((SLOTS, EH), np.float32)
        efs[pos] = ef

        ecb = np.zeros((NCHE, 17, 128), np.float16)
        ecb[:NCH, :16, :] = efs.reshape(NCH, 128, EH).transpose(0, 2, 1)
        ecb[:NCH, 16, :] = 1.0
        ecb2 = (ecb.reshape(NCHE // 2, 2, 17, 128).transpose(0, 2, 1, 3)
                .reshape(NCHE // 2, 17, 256))

        # dma_gather index layout: idx j at [j%16, j//16], replicated x8
        idx16 = srcs.reshape(SLOTS // 16, 16).T
        idx128 = np.tile(idx16, (8, 1)).astype(np.int16)

        ch = np.arange(SLOTS) // 128
        ei = np.arange(SLOTS) % 128
        valid = rv >= 0
        ri = np.where(valid, rv, 0).astype(np.int64)
        scb = np.zeros((NCHE, 128, 128), np.float16)
        sctb = np.zeros((NCHE, 128, 128), np.float16)
        scb[ch[valid], ri[valid], ei[valid]] = 1.0
        sctb[ch[valid], ei[valid], ri[valid]] = 1.0
        scb2 = (scb.reshape(NCHE // 2, 2, 128, 128).transpose(0, 2, 1, 3)
                .reshape(NCHE // 2, 128, 256))
        sctb2 = (sctb.reshape(NCHE // 2, 2, 128, 128).transpose(0, 2, 1, 3)
                 .reshape(NCHE // 2, 128, 256))

        gmask = np.zeros((NT, 128, B), np.float16)
        nodes_c = np.where(core_of_node == c)[0]
        gmask[grp_in_core[nodes_c], slot_of[nodes_c], batch[nodes_c]] = 1.0

        xT = np.zeros((KX * 128, NLOC), np.float16)
        own = grp_in_core[nodes_c] * 128 + slot_of[nodes_c]
        xT[:NODE_DIM, own] = x[nodes_c].T.astype(np.float16)
        xT[NODE_DIM, :] = 1.0  # bias row

        per_core.append(dict(xT=xT, ecb=ecb2, idx=idx128, scb=scb2,
                             sctb=sctb2, gmask=gmask))
        dbg.append(dict(srcs=srcs, rv=rv, efs=efs))

    # ---- shared weights
    f32 = np.float32
    W_in = np.asarray(inputs["W_in"], f32)
    b_in = np.asarray(inputs["b_in"], f32)
    winp = np.zeros((KX * 128, HID), f32)
    winp[:NODE_DIM] = W_in
    winp[NODE_DIM] = b_in
    winp = winp.reshape(KX, 128, HID).astype(np.float16)

    def rep(v):  # replicate a [HID] vector across partitions
        return np.broadcast_to(np.asarray(v, f32), (128, HID)).astype(np.float16).copy()

    Wl = np.asarray(inputs["Wl"], f32)
    Wr = np.asarray(inputs["Wr"], f32)
    bl = np.asarray(inputs["bl"], f32)
    br = np.asarray(inputs["br"], f32)
    We = np.asarray(inputs["We"], f32)
    att = np.asarray(inputs["att"], f32)
    bconv = np.asarray(inputs["bconv"], f32)
    ln_g = np.asarray(inputs["ln_g"], f32)
    ln_b = np.asarray(inputs["ln_b"], f32)

    wl = Wl.reshape(L, 2, 128, HID).astype(np.float16)
    wr = Wr.reshape(L, 2, 128, HID).astype(np.float16)
    weaug = np.zeros((L, 17, HID), f32)
    weaug[:, :16] = We
    weaug[:, 16] = bl + br
    weaug = weaug.astype(np.float16)
    attf = np.stack([rep(att[i].reshape(HID)) for i in range(L)])
    bconv2 = np.stack([rep(bconv[i] + bl[i]) for i in range(L)])
    lng = np.stack([rep(ln_g[i]) for i in range(L)])
    lnb = np.stack([rep(ln_b[i]) for i in range(L)])

    Wg1 = np.asarray(inputs["Wg1"], f32)  # (256,128)
    Wg2 = np.asarray(inputs["Wg2"], f32)  # (128,1)
    Wh1 = np.asarray(inputs["Wh1"], f32)  # (256,64)
    Wh2 = np.asarray(inputs["Wh2"], f32)  # (64,1)
    GW = Wg1.shape[1]
    HW1 = Wh1.shape[1]
    shared = dict(
        winp=winp,
        lnin_g=rep(inputs["ln_in_g"]), lnin_b=rep(inputs["ln_in_b"]),
        wl=wl, wr=wr, weaug=weaug, attf=attf, bconv2=bconv2, lng=lng, lnb=lnb,
        wg1=Wg1.reshape(2, 128, GW).astype(np.float16),
        bg1=np.asarray(inputs["bg1"], f32).reshape(1, GW).astype(np.float16),
        wg2=np.broadcast_to(Wg2.reshape(GW), (128, GW)).astype(np.float16).copy(),
        bg2=np.full((128, 1), float(np.asarray(inputs["bg2"]).reshape(())), f32),
        wh1=Wh1.reshape(2, 128, HW1).astype(np.float16),
        bh1=np.broadcast_to(np.asarray(inputs["bh1"], f32), (128, HW1)).astype(np.float16).copy(),
        wh2=np.broadcast_to(Wh2.reshape(HW1), (128, HW1)).astype(np.float16).copy(),
        bh2=np.full((128, 1), float(np.asarray(inputs["bh2"]).reshape(())), f32),
    )

    in_maps = []
    for c in range(NCORES):
        m = dict(shared)
        m.update(per_core[c])
        in_maps.append({k: np.ascontiguousarray(v) for k, v in m.items()})

    meta = dict(NLOC=NLOC, NT=NT, CPG=CPG, NCH=NCH, NCHE=NCHE, SLOTS=SLOTS,
                GLOB=GLOB, GW=GW, HW1=HW1, in_maps=in_maps, dbg=dbg)
    return meta


# ----------------------------------------------------------------------------
# device program
# ----------------------------------------------------------------------------
def build(meta, num_devices=NCORES, nlayers=L):
    NLOC, NT, CPG, NCH = meta["NLOC"], meta["NT"], meta["CPG"], meta["NCH"]
    NCHE, SLOTS, GW, HW1 = meta["NCHE"], meta["SLOTS"], meta["GW"], meta["HW1"]
    ICOLS = SLOTS // 16
    SUP = 8  # chunks per supergather (dma_gather fails above 1024 idxs/call)

    nc = bacc.Bacc("TRN2", target_bir_lowering=False, debug=False,
                   enable_asserts=True, num_devices=num_devices)

    def din(name, shape, dt=F16):
        return nc.dram_tensor(name, list(shape), dt, kind="ExternalInput").ap()

    # inputs
    xT_d = din("xT", (KX * 128, NLOC))
    winp_d = din("winp", (KX, 128, HID))
    lnin_g_d = din("lnin_g", (128, HID))
    lnin_b_d = din("lnin_b", (128, HID))
    wl_d = din("wl", (L, 2, 128, HID))
    wr_d = din("wr", (L, 2, 128, HID))
    weaug_d = din("weaug", (L, 17, HID))
    attf_d = din("attf", (L, 128, HID))
    bconv2_d = din("bconv2", (L, 128, HID))
    lng_d = din("lng", (L, 128, HID))
    lnb_d = din("lnb", (L, 128, HID))
    ecb_d = din("ecb", (NCHE // 2, 17, 256))
    idx_d = din("idx", (128, ICOLS), I16)
    scb_d = din("scb", (NCHE // 2, 128, 256))
    sctb_d = din("sctb", (NCHE // 2, 128, 256))
    gmask_d = din("gmask", (NT, 128, B))
    wg1_d = din("wg1", (2, 128, GW))
    bg1_d = din("bg1", (1, GW))
    wg2_d = din("wg2", (128, GW))
    bg2_d = din("bg2", (128, 1), F32)
    wh1_d = din("wh1", (2, 128, HW1))
    bh1_d = din("bh1", (128, HW1))
    wh2_d = din("wh2", (128, HW1))
    bh2_d = din("bh2", (128, 1), F32)
    y_d = nc.dram_tensor("y", [B, 1], F32, kind="ExternalOutput").ap()
    import os
    DEBUG = bool(os.environ.get("GNN_DEBUG"))
    VAR = os.environ.get("GNN_VAR", "")
    if DEBUG:
        dbg_xlg_d = nc.dram_tensor("dbg_xlg", [128, 8, HID], F16,
                                   kind="ExternalOutput").ap()
        dbg_h_d = nc.dram_tensor("dbg_h", [NLOC, HID], F16,
                                 kind="ExternalOutput").ap()
        dbg_pp_d = nc.dram_tensor("dbg_pp", [128, 48], F32,
                                  kind="ExternalOutput").ap()
        dbg_pq_d = nc.dram_tensor("dbg_pq", [128, 48], F32,
                                  kind="ExternalOutput").ap()
        dbg_eg_d = nc.dram_tensor("dbg_eg", [128, NT], F32,
                                  kind="ExternalOutput").ap()

    # internal DRAM
    hloc_d = nc.dram_tensor("hloc", [NLOC, HID], F16).ap()
    xlb_d = nc.dram_tensor("xlb", [NLOC, HID], F16).ap()
    xla_d = [nc.dram_tensor(f"xla{i}", [NCORES * NLOC, HID], F16,
                            addr_space="Shared").ap() for i in range(nlayers)]
    poolb_d = nc.dram_tensor("poolb", [128, 48], F32).ap()
    pools_d = nc.dram_tensor("pools", [128, 48], F32,
                             addr_space="Shared").ap()

    rg = [list(range(num_devices))]

    with tile.TileContext(nc) as tc:
        import contextlib
        ctx = contextlib.ExitStack()
        with ctx:
            const = ctx.enter_context(tc.tile_pool(name="const", bufs=1))
            work = ctx.enter_context(tc.tile_pool(name="work", bufs=3))
            small = ctx.enter_context(tc.tile_pool(name="small", bufs=4))
            xtp = ctx.enter_context(tc.tile_pool(name="xtp", bufs=3))
            scp = ctx.enter_context(tc.tile_pool(name="scp", bufs=3))
            xlg_p = ctx.enter_context(tc.tile_pool(name="xlg", bufs=2))
            ps_mm = ctx.enter_context(tc.tile_pool(name="ps_mm", bufs=2, space="PSUM"))
            ps_ed = ctx.enter_context(tc.tile_pool(name="ps_ed", bufs=2, space="PSUM"))
            ps_ag = ctx.enter_context(tc.tile_pool(name="ps_ag", bufs=2, space="PSUM"))

            # ---------------- resident tiles
            winp_t = const.tile([128, KX, HID], F16)
            nc.sync.dma_start(out=winp_t[:], in_=winp_d.rearrange("k p f -> p k f"))
            lnin_g_t = const.tile([128, HID], F16)
            nc.sync.dma_start(out=lnin_g_t[:], in_=lnin_g_d[:])
            lnin_b_t = const.tile([128, HID], F16)
            nc.sync.dma_start(out=lnin_b_t[:], in_=lnin_b_d[:])
            wl_t = const.tile([128, L, 2, HID], F16)
            nc.sync.dma_start(out=wl_t[:], in_=wl_d.rearrange("l k p f -> p l k f"))
            wr_t = const.tile([128, L, 2, HID], F16)
            nc.sync.dma_start(out=wr_t[:], in_=wr_d.rearrange("l k p f -> p l k f"))
            weaug_t = const.tile([17, L, HID], F16)
            nc.sync.dma_start(out=weaug_t[:], in_=weaug_d.rearrange("l p f -> p l f"))
            attf_t = const.tile([128, L, HID], F16)
            nc.sync.dma_start(out=attf_t[:], in_=attf_d.rearrange("l p f -> p l f"))
            bconv2_t = const.tile([128, L, HID], F16)
            nc.sync.dma_start(out=bconv2_t[:], in_=bconv2_d.rearrange("l p f -> p l f"))
            lng_t = const.tile([128, L, HID], F16)
            nc.sync.dma_start(out=lng_t[:], in_=lng_d.rearrange("l p f -> p l f"))
            lnb_t = const.tile([128, L, HID], F16)
            nc.sync.dma_start(out=lnb_t[:], in_=lnb_d.rearrange("l p f -> p l f"))
            idx_t = const.tile([128, ICOLS], I16)
            nc.sync.dma_start(out=idx_t[:], in_=idx_d[:])
            gmask_t = const.tile([128, NT, B], F16)
            nc.sync.dma_start(out=gmask_t[:], in_=gmask_d.rearrange("t p g -> p t g"))
            wg1_t = const.tile([128, 2, GW], F16)
            nc.sync.dma_start(out=wg1_t[:], in_=wg1_d.rearrange("k p f -> p k f"))
            bg1_t = const.tile([1, GW], F16)
            nc.sync.dma_start(out=bg1_t[:], in_=bg1_d[:])
            wg2_t = const.tile([128, GW], F16)
            nc.sync.dma_start(out=wg2_t[:], in_=wg2_d[:])
            bg2_t = const.tile([128, 1], F32)
            nc.sync.dma_start(out=bg2_t[:], in_=bg2_d[:])
            wh1_t = const.tile([128, 2, HW1], F16)
            nc.sync.dma_start(out=wh1_t[:], in_=wh1_d.rearrange("k p f -> p k f"))
            bh1_t = const.tile([128, HW1], F16)
            nc.sync.dma_start(out=bh1_t[:], in_=bh1_d[:])
            wh2_t = const.tile([128, HW1], F16)
            nc.sync.dma_start(out=wh2_t[:], in_=wh2_d[:])
            bh2_t = const.tile([128, 1], F32)
            nc.sync.dma_start(out=bh2_t[:], in_=bh2_d[:])

            h_res = const.tile([128, NT, HID + 1], F16)
            hT_loc = const.tile([128, 2, NLOC], F16)
            xr_t = const.tile([128, NT, HID], F16)
            ones1_t = const.tile([1, 128], F16)
            nc.vector.memset(ones1_t[:], 1.0)
            eps_t = const.tile([128, 1], F32)
            nc.vector.memset(eps_t[:], 1e-5)
            expb_t = const.tile([128, 1], F32)
            nc.vector.memset(expb_t[:], EXP_BIAS)
            gateb_t = const.tile([128, 1], F32)
            nc.vector.memset(gateb_t[:], GATE_BIAS)
            for t in range(NT):
                nc.vector.memset(h_res[:, t, HID:HID + 1], 1.0)

            def refine_recip(r_ap, x_ap, shape, tag):
                # r <- r*(2 - x*r), one Newton step on a LUT seed
                t = small.tile(shape, F32, tag=tag)
                nc.vector.tensor_tensor(out=t[:], in0=x_ap, in1=r_ap, op=OP.mult)
                nc.vector.tensor_scalar(out=t[:], in0=t[:], scalar1=2.0,
                                        scalar2=-1.0, op0=OP.subtract, op1=OP.mult)
                nc.vector.tensor_tensor(out=r_ap, in0=r_ap, in1=t[:], op=OP.mult)

            def refine_rsqrt(r_ap, x_ap, shape, tag):
                # r <- 0.5*r*(3 - x*r*r)
                t = small.tile(shape, F32, tag=tag)
                nc.vector.tensor_tensor(out=t[:], in0=r_ap, in1=r_ap, op=OP.mult)
                nc.vector.tensor_tensor(out=t[:], in0=x_ap, in1=t[:], op=OP.mult)
                nc.vector.tensor_scalar(out=t[:], in0=t[:], scalar1=3.0,
                                        scalar2=-0.5, op0=OP.subtract, op1=OP.mult)
                nc.vector.tensor_tensor(out=r_ap, in0=r_ap, in1=t[:], op=OP.mult)

            # ---------------- LN helper: s_t fp16 [128,HID] + musum f32 -> dest
            def layernorm(s_t, musum, g_ap, b_ap, dest_ap, gelu_after=False):
                mu = small.tile([128, 1], F32, tag="mu")
                nc.vector.tensor_scalar(out=mu[:], in0=musum, scalar1=1.0 / HID,
                                        scalar2=None, op0=OP.mult)
                d_t = work.tile([128, HID], F16, tag="d")
                nc.vector.tensor_scalar(out=d_t[:], in0=s_t, scalar1=mu[:],
                                        scalar2=None, op0=OP.subtract)
                scr = work.tile([128, HID], F16, tag="scr")
                vs = small.tile([128, 1], F32, tag="vs")
                nc.vector.tensor_tensor(out=scr[:], in0=d_t[:], in1=d_t[:], op=OP.mult)
                nc.vector.tensor_reduce(out=vs[:], in_=scr[:],
                                        axis=mybir.AxisListType.X, op=OP.add)
                vx = small.tile([128, 1], F32, tag="vx")
                nc.vector.tensor_scalar(out=vx[:], in0=vs[:], scalar1=1.0 / HID,
                                        scalar2=None, op0=OP.mult)
                nc.vector.tensor_scalar(out=vx[:], in0=vx[:], scalar1=eps_t[:],
                                        scalar2=None, op0=OP.add)
                sd = small.tile([128, 1], F32, tag="sd")
                nc.scalar.activation(out=sd[:], in_=vx[:], func=AF.Ln)
                rstd = small.tile([128, 1], F32, tag="rstd")
                nc.scalar.activation(out=rstd[:], in_=sd[:], func=AF.Exp, scale=-0.5)
                refine_rsqrt(rstd[:], vx[:], [128, 1], "nsr")
                n_t = work.tile([128, HID], F16, tag="n")
                nc.vector.tensor_scalar(out=n_t[:], in0=d_t[:], scalar1=rstd[:],
                                        scalar2=None, op0=OP.mult)
                nc.vector.tensor_tensor(out=n_t[:], in0=n_t[:], in1=g_ap, op=OP.mult)
                if gelu_after:
                    nc.vector.tensor_tensor(out=n_t[:], in0=n_t[:], in1=b_ap, op=OP.add)
                    nc.scalar.activation(out=dest_ap, in_=n_t[:], func=AF.Gelu)
                else:
                    nc.vector.tensor_tensor(out=dest_ap, in0=n_t[:], in1=b_ap, op=OP.add)

            # ---------------- phase A: input projection (local nodes)
            for t2 in range((NT + 1) // 2):
                tcnt = min(2, NT - t2 * 2)
                xt_t = xtp.tile([128, KX, 2 * 128], F16, tag="xt")
                for k in range(KX):
                    nc.sync.dma_start(
                        out=xt_t[:, k, :tcnt * 128],
                        in_=xT_d[k * 128:(k + 1) * 128,
                                 t2 * 256:t2 * 256 + tcnt * 128])
                for j in range(tcnt):
                    t = t2 * 2 + j
                    ps = ps_mm.tile([128, HID], F32, tag="mmps")
                    for k in range(KX):
                        nc.tensor.matmul(out=ps[:], lhsT=xt_t[:, k, j * 128:(j + 1) * 128],
                                         rhs=winp_t[:, k, :], start=(k == 0),
                                         stop=(k == KX - 1))
                    s_t = work.tile([128, HID], F16, tag="s")
                    musum = small.tile([128, 1], F32, tag="musum")
                    nc.scalar.activation(out=s_t[:], in_=ps[:], func=AF.Copy,
                                         accum_out=musum[:])
                    layernorm(s_t[:], musum[:], lnin_g_t[:], lnin_b_t[:],
                              h_res[:, t, :HID], gelu_after=True)
                    nc.sync.dma_start(out=hloc_d[t * 128:(t + 1) * 128, :],
                                      in_=h_res[:, t, :HID])

            # ---------------- per layer
            def compute_hT():
                # hloc (node-major, HBM) -> hT_loc (feature-major, SBUF)
                for half in range(2):
                    nc.sync.dma_start(out=hT_loc[:, half, :],
                                      in_=hloc_d[:, half * 128:(half + 1) * 128],
                                      transpose=True)

            compute_hT()

            for i in range(nlayers):
                # local xl -> bounce -> AllGather into shared node-major table
                for t in range(NT):
                    ps = ps_mm.tile([128, HID], F32, tag="mmps")
                    for half in range(2):
                        nc.tensor.matmul(out=ps[:],
                                         lhsT=hT_loc[:, half, t * 128:(t + 1) * 128],
                                         rhs=wl_t[:, i, half, :],
                                         start=(half == 0), stop=(half == 1))
                    xl_t = work.tile([128, HID], F16, tag="xlt")
                    nc.scalar.activation(out=xl_t[:], in_=ps[:], func=AF.Copy)
                    nc.sync.dma_start(out=xlb_d[t * 128:(t + 1) * 128, :],
                                      in_=xl_t[:])
                if VAR != "nocoll":
                    nc.gpsimd.collective_compute(
                        "AllGather", OP.bypass, replica_groups=rg,
                        ins=[xlb_d[:]], outs=[xla_d[i][:]])
                # local xr (overlaps the collective)
                for t in range(NT):
                    ps = ps_mm.tile([128, HID], F32, tag="mmps")
                    for half in range(2):
                        nc.tensor.matmul(out=ps[:],
                                         lhsT=hT_loc[:, half, t * 128:(t + 1) * 128],
                                         rhs=wr_t[:, i, half, :],
                                         start=(half == 0), stop=(half == 1))
                    nc.scalar.activation(out=xr_t[:, t, :], in_=ps[:], func=AF.Copy)

                # edge phase
                xlg_tiles = {}
                agg = None
                for chk in range(NCH):
                    s, joff = divmod(chk, SUP)
                    if VAR == "nogather" and s > 0:
                        s = 0
                    if joff == 0 and (VAR != "nogather" or s == 0 or True) and not (VAR == "nogather" and s != chk // SUP):
                        cnt = min(SUP, NCH - s * SUP)
                        xlg = xlg_p.tile([128, SUP, HID], F16, tag="xlg")
                        nc.gpsimd.dma_gather(
                            out_ap=xlg[:, :cnt, :], in_ap=xla_d[i][:, :],
                            idxs_ap=idx_t[:, s * (SUP * 8):s * (SUP * 8) + cnt * 8],
                            num_idxs=cnt * 128, num_idxs_reg=cnt * 128,
                            elem_size=HID)
                        xlg_tiles[s] = xlg
                        if DEBUG and i == 0 and s == 0:
                            nc.sync.dma_start(out=dbg_xlg_d[:], in_=xlg[:, :8, :])
                    xlg = xlg_tiles[s]
                    g, cidx = divmod(chk, CPG)

                    if chk % 2 == 0 and (VAR != "noec" or chk == 0):
                        ec2 = scp.tile([17, 256], F16, tag="ec2")
                        nc.sync.dma_start(out=ec2[:], in_=ecb_d[chk // 2])
                        ec2_cur = ec2
                    half = (chk % 2) * 128

                    if chk % 2 == 0 and (VAR != "nosel" or chk == 0):
                        sc2 = scp.tile([128, 256], F16, tag="sc2")
                        nc.sync.dma_start(out=sc2[:], in_=scb_d[chk // 2])
                        sct2 = scp.tile([128, 256], F16, tag="sct2")
                        nc.sync.dma_start(out=sct2[:], in_=sctb_d[chk // 2])
                        sc2_cur, sct2_cur = sc2, sct2
                    scb_t = sc2_cur[:, half:half + 128]
                    sct_t = sct2_cur[:, half:half + 128]

                    do_mm = VAR != "nomm" or chk == 0
                    do_vec = VAR != "novec" or chk == 0
                    if do_mm:
                        ps = ps_ed.tile([128, HID], F32, tag="edps")
                        nc.tensor.matmul(out=ps[:], lhsT=scb_t,
                                         rhs=xr_t[:, g, :], start=True, stop=False)
                        nc.tensor.matmul(out=ps[:], lhsT=ec2_cur[:, half:half + 128],
                                         rhs=weaug_t[:, i, :], start=False, stop=True)
                        ps_keep = ps
                    else:
                        ps = ps_keep
                    m_t = work.tile([128, HID], F16, tag="m")
                    if do_vec:
                        nc.vector.tensor_tensor(out=m_t[:], in0=xlg[:, joff, :],
                                                in1=ps[:], op=OP.add)
                    if do_vec:
                        lr_t = work.tile([128, HID], F16, tag="lr")
                        nc.scalar.activation(out=lr_t[:], in_=m_t[:], func=AF.Copy,
                                             scale=0.2)
                        nc.vector.tensor_tensor(out=m_t[:], in0=m_t[:], in1=lr_t[:],
                                                op=OP.max)
                        v_t = work.tile([128, HID], F16, tag="v")
                        nc.vector.tensor_tensor(out=v_t[:], in0=m_t[:],
                                                in1=attf_t[:, i, :], op=OP.mult)
                        a_t = small.tile([128, H], F32, tag="a")
                        nc.vector.tensor_reduce(
                            out=a_t[:], in_=v_t[:].rearrange("p (h d) -> p h d", d=DH),
                            axis=mybir.AxisListType.X, op=OP.add)
                        u_t = work.tile([128, HID + H], F16, tag="u")
                        nc.scalar.activation(out=u_t[:, HID:HID + H], in_=a_t[:],
                                             func=AF.Exp, bias=expb_t[:])
                        nc.vector.tensor_tensor(
                            out=u_t[:, :HID].rearrange("p (h d) -> p h d", d=DH),
                            in0=xlg[:, joff, :].rearrange("p (h d) -> p h d", d=DH),
                            in1=u_t[:, HID:HID + H].to_broadcast([128, H, DH]),
                            op=OP.mult)
                        u_keep = u_t
                    else:
                        u_t = u_keep
                    if cidx == 0:
                        agg = ps_ag.tile([128, HID + H], F32, tag="agg")
                    if do_mm:
                        nc.tensor.matmul(out=agg[:], lhsT=sct_t,
                                         rhs=u_t[:], start=(cidx == 0),
                                         stop=(cidx == CPG - 1))

                    if cidx == CPG - 1:
                        rd = small.tile([128, H], F32, tag="rd")
                        nc.scalar.activation(out=rd[:], in_=agg[:, HID:HID + H],
                                             func=AF.Ln)
                        nc.scalar.activation(out=rd[:], in_=rd[:], func=AF.Exp,
                                             scale=-1.0)
                        refine_recip(rd[:], agg[:, HID:HID + H], [128, H], "nrd")
                        o_t = work.tile([128, HID], F16, tag="o")
                        nc.vector.tensor_tensor(
                            out=o_t[:].rearrange("p (h d) -> p h d", d=DH),
                            in0=agg[:, :HID].rearrange("p (h d) -> p h d", d=DH),
                            in1=rd[:].to_broadcast([128, H, DH]), op=OP.mult)
                        nc.vector.tensor_tensor(out=o_t[:], in0=o_t[:],
                                                in1=bconv2_t[:, i, :], op=OP.add)
                        nc.scalar.activation(out=o_t[:], in_=o_t[:], func=AF.Gelu)
                        s_t = work.tile([128, HID], F16, tag="s")
                        musum = small.tile([128, 1], F32, tag="musum")
                        nc.vector.tensor_tensor(out=s_t[:], in0=o_t[:],
                                                in1=h_res[:, g, :HID], op=OP.add)
                        nc.vector.tensor_reduce(out=musum[:], in_=s_t[:],
                                                axis=mybir.AxisListType.X, op=OP.add)
                        layernorm(s_t[:], musum[:], lng_t[:, i, :], lnb_t[:, i, :],
                                  h_res[:, g, :HID])
                        nc.sync.dma_start(out=hloc_d[g * 128:(g + 1) * 128, :],
                                          in_=h_res[:, g, :HID])
                compute_hT()

            if DEBUG:
                for t in range(NT):
                    hcp = work.tile([128, HID], F16, tag="hcp")
                    nc.sync.dma_start(out=hcp[:],
                                      in_=hloc_d[t * 128:(t + 1) * 128, :])
                    nc.sync.dma_start(out=dbg_h_d[t * 128:(t + 1) * 128, :],
                                      in_=hcp[:])

            # ---------------- pooling (transposed accumulation) + head
            wm_all = const.tile([128, NT, B], F16)
            for t in range(NT):
                g1 = ps_mm.tile([128, HID], F32, tag="mmps")
                for half in range(2):
                    nc.tensor.matmul(out=g1[:, :GW],
                                     lhsT=hT_loc[:, half, t * 128:(t + 1) * 128],
                                     rhs=wg1_t[:, half, :], start=(half == 0),
                                     stop=False)
                nc.tensor.matmul(out=g1[:, :GW], lhsT=ones1_t[:],
                                 rhs=bg1_t[:], start=False, stop=True)
                t_t = work.tile([128, GW], F16, tag="tt")
                nc.scalar.activation(out=t_t[:], in_=g1[:, :GW], func=AF.Tanh)
                scr = work.tile([128, GW], F16, tag="scr2")
                gate = small.tile([128, 1], F32, tag="gate")
                nc.vector.tensor_tensor(out=scr[:], in0=t_t[:], in1=wg2_t[:],
                                        op=OP.mult)
                nc.vector.tensor_reduce(out=gate[:], in_=scr[:],
                                        axis=mybir.AxisListType.X, op=OP.add)
                nc.vector.tensor_scalar(out=gate[:], in0=gate[:], scalar1=bg2_t[:],
                                        scalar2=None, op0=OP.add)
                eg = small.tile([128, 1], F16, tag="eg")
                nc.scalar.activation(out=eg[:], in_=gate[:], func=AF.Exp,
                                     bias=gateb_t[:])
                if DEBUG:
                    egd = small.tile([128, 1], F32, tag="egd")
                    nc.vector.tensor_scalar(out=egd[:], in0=eg[:], scalar1=1.0,
                                            scalar2=None, op0=OP.mult)
                    nc.sync.dma_start(out=dbg_eg_d[:, t:t + 1], in_=egd[:])
                nc.vector.tensor_tensor(out=wm_all[:, t, :], in0=gmask_t[:, t, :],
                                        in1=eg[:].to_broadcast([128, B]), op=OP.mult)
            # three sequential single-group accumulation passes: a matmul
            # start=True resets its whole PSUM bank, so groups must not
            # interleave within a bank.
            pp = work.tile([128, 48], F32, tag="pp")
            nc.vector.memset(pp[:], 0.0)
            lhss = [(slice(0, 128), (0, 16), 128),
                    (slice(128, 256), (16, 32), 128),
                    (slice(HID, HID + 1), (32, 48), 1)]
            for lh, (c0, c1), mrows in lhss:
                psp = ps_ag.tile([128, HID + H], F32, tag="agg")
                for t in range(NT):
                    nc.tensor.matmul(out=psp[:mrows, 0:16],
                                     lhsT=h_res[:, t, lh],
                                     rhs=wm_all[:, t, :],
                                     start=(t == 0), stop=(t == NT - 1))
                nc.scalar.activation(out=pp[:mrows, c0:c1], in_=psp[:mrows, 0:16],
                                     func=AF.Copy)
            nc.sync.dma_start(out=poolb_d[:], in_=pp[:])
            if DEBUG:
                nc.sync.dma_start(out=dbg_pp_d[:], in_=pp[:])
            nc.gpsimd.collective_compute(
                "AllReduce", OP.add, replica_groups=rg,
                ins=[poolb_d[:]], outs=[pools_d[:]])
            pq = work.tile([128, 48], F32, tag="pq")
            nc.sync.dma_start(out=pq[:], in_=pools_d[:])
            if DEBUG:
                nc.sync.dma_start(out=dbg_pq_d[:], in_=pq[:])
            # reciprocal of the per-graph denominators (row 0, cols 32:48)
            rdp = small.tile([1, 16], F32, tag="rdp")
            nc.scalar.activation(out=rdp[:], in_=pq[0:1, 32:48], func=AF.Ln)
            nc.scalar.activation(out=rdp[:], in_=rdp[:], func=AF.Exp, scale=-1.0)
            refine_recip(rdp[:], pq[0:1, 32:48], [1, 16], "nrp")
            rdb = small.tile([128, 16], F32, tag="rdb")
            nc.gpsimd.partition_broadcast(rdb[:], rdp[:])
            pooledT = work.tile([128, 2, 16], F16, tag="pooledT")
            for half in range(2):
                nc.vector.tensor_tensor(out=pooledT[:, half, :],
                                        in0=pq[:, half * 16:(half + 1) * 16],
                                        in1=rdb[:], op=OP.mult)
            o1ps = ps_mm.tile([128, HID], F32, tag="mmps")
            for half in range(2):
                nc.tensor.matmul(out=o1ps[0:16, 0:HW1], lhsT=pooledT[:, half, :],
                                 rhs=wh1_t[:, half, :], start=(half == 0),
                                 stop=(half == 1))
            o1 = work.tile([16, HW1], F16, tag="o1s")
            nc.vector.tensor_tensor(out=o1[:], in0=o1ps[0:16, 0:HW1],
                                    in1=bh1_t[0:16, :], op=OP.add)
            nc.scalar.activation(out=o1[:], in_=o1[:], func=AF.Gelu)
            scr3 = work.tile([16, HW1], F16, tag="scr3")
            yv = small.tile([16, 1], F32, tag="yv")
            nc.vector.tensor_tensor(out=scr3[:], in0=o1[:], in1=wh2_t[0:16, :],
                                    op=OP.mult)
            nc.vector.tensor_reduce(out=yv[:], in_=scr3[:],
                                    axis=mybir.AxisListType.X, op=OP.add)
            nc.vector.tensor_scalar(out=yv[:], in0=yv[:], scalar1=bh2_t[0:16, :],
                                    scalar2=None, op0=OP.add)
            nc.sync.dma_start(out=y_d[:], in_=yv[:])

    nc.compile()
    return nc


# ----------------------------------------------------------------------------
# persistent execution layer: compile once, keep inputs device-resident, so
# repeat executions measure kernel time rather than PJRT re-trace + re-stage.
# Mirrors concourse.bass2jax.run_bass_via_pjrt's lowering contract exactly.
# ----------------------------------------------------------------------------
def _make_runner(nc, in_maps, n_cores):
    import jax
    from jax.experimental.shard_map import shard_map
    from jax.sharding import Mesh, PartitionSpec, NamedSharding
    from concourse import bass2jax

    bass2jax.install_neuronx_cc_hook()

    if nc.dbg_addr is not None:
        if nc.dbg_callbacks:
            raise RuntimeError("dbg callbacks unsupported in persistent runner")
        in_maps = [{**m, nc.dbg_addr.name: np.zeros((1, 2), np.uint32)}
                   for m in in_maps]

    partition_name = nc.partition_id_tensor.name if nc.partition_id_tensor else None
    in_names, out_names, out_avals, zero_outs = [], [], [], []
    for alloc in nc.m.functions[0].allocations:
        if not isinstance(alloc, mybir.MemoryLocationSet):
            continue
        name = alloc.memorylocations[0].name
        if alloc.kind == "ExternalInput":
            if name != partition_name:
                in_names.append(name)
        elif alloc.kind == "ExternalOutput":
            shape = tuple(alloc.tensor_shape)
            dtype = mybir.dt.np(alloc.dtype)
            out_names.append(name)
            out_avals.append(jax.core.ShapedArray(shape, dtype))
            zero_outs.append(np.zeros(shape, dtype))
    n_params = len(in_names)
    n_outs = len(out_avals)
    in_names_all = list(in_names) + out_names
    if partition_name is not None:
        in_names_all.append(partition_name)

    def _body(*args):
        operands = list(args)
        if partition_name is not None:
            operands.append(bass2jax.partition_id_tensor())
        outs = bass2jax._bass_exec_p.bind(
            *operands,
            out_avals=tuple(out_avals),
            in_names=tuple(in_names_all),
            out_names=tuple(out_names),
            lowering_input_output_aliases=(),
            sim_require_finite=True,
            sim_require_nnan=True,
            nc=nc,
        )
        return tuple(outs)

    devices = jax.devices()[:n_cores]
    mesh = Mesh(np.asarray(devices), ("core",))
    in_specs = (PartitionSpec("core"),) * (n_params + n_outs)
    out_specs = (PartitionSpec("core"),) * n_outs
    fn = shard_map(_body, mesh=mesh, in_specs=in_specs, out_specs=out_specs,
                   check_rep=False)

    per_core = [[np.asarray(m[name]) for name in in_names] for m in in_maps]
    concat_in = [np.concatenate([per_core[c][i] for c in range(n_cores)], axis=0)
                 for i in range(n_params)]
    sh = NamedSharding(mesh, PartitionSpec("core"))
    dev_in = [jax.device_put(a, sh) for a in concat_in]

    # No donation: the kernel fully writes its ExternalOutputs, so the
    # zero "output seed" buffers can stay resident across calls.
    zshapes = [(n_cores * z.shape[0], *z.shape[1:]) for z in zero_outs]
    zdtypes = [z.dtype for z in zero_outs]
    dev_zeros = [jax.device_put(np.zeros(s, d), sh)
                 for s, d in zip(zshapes, zdtypes)]
    compiled = bass2jax.fast_dispatch_compile(
        lambda: jax.jit(fn, keep_unused=True)
        .lower(*dev_in, *dev_zeros).compile())
    global _LAST_RUNNER
    _LAST_RUNNER = (compiled, dev_in, dev_zeros)

    def run():
        outs = jax.block_until_ready(compiled(*dev_in, *dev_zeros))
        return [
            {name: np.asarray(outs[i]).reshape(n_cores, *out_avals[i].shape)[c]
             for i, name in enumerate(out_names)}
            for c in range(n_cores)
        ]
    return run


# ----------------------------------------------------------------------------
# entry point
# ----------------------------------------------------------------------------
LAST_EXEC_NS = None
_LAST = {}
_LAST_RUNNER = None


def rerun(n=3):
    """Re-execute the already-built program; returns min wall seconds."""
    import time
    run = _LAST["run"]
    best = float("inf")
    for _ in range(n):
        t0 = time.time()
        run()
        best = min(best, time.time() - t0)
    return best


def exec_time_s(k=100, warm=3):
    """Steady-state per-execution time: marginal wall of pipelined launches."""
    import time
    import jax
    compiled, dev_in, dev_zeros = _LAST_RUNNER
    for _ in range(warm):
        jax.block_until_ready(compiled(*dev_in, *dev_zeros))
    t0 = time.time()
    outs = None
    for _ in range(k):
        outs = compiled(*dev_in, *dev_zeros)
    jax.block_until_ready(outs)
    t_k = time.time() - t0
    t0 = time.time()
    outs = None
    for _ in range(2 * k):
        outs = compiled(*dev_in, *dev_zeros)
    jax.block_until_ready(outs)
    t_2k = time.time() - t0
    return max((t_2k - t_k) / k, 1e-9)


def kernel(**inputs):
    global LAST_EXEC_NS
    from concourse.bass_interp import get_hw_module

    meta = prepare(inputs)
    nc = build(meta)
    nc.m = get_hw_module(nc.m)
    run = _make_runner(nc, meta["in_maps"], NCORES)
    results = run()
    _LAST.update(nc=nc, meta=meta, run=run)
    return results[0]["y"].reshape(B).astype(np.float32).copy()
